# revision 41
# baseline (speedup 1.0000x reference)
"""Adjacency-aware multi-head attention on 8 trn2 NeuronCores.

Math (per b, head k):
  Q = h[b] @ Wq[:, k] + bq[k]           [N, D]
  S[i, j] = (Q_i . K_j) / sqrt(D)
  P[j, i] = exp(S[i, j]) / sum_j exp(S[i, j])      (softmax over keys j)
  out[i, d] = sum_j P[j, i] * A[b, j, i] * V[j, d]

The K bias cancels: it adds g[i] = Q_i . bk to every score of query i,
and softmax over j is invariant to per-i shifts -> bk is dropped.

Sharding: 16 (b, head) pairs over 8 cores, 2 heads of the SAME b per core so
the A[b] stream is shared by both heads.

Device dataflow ([j, i] layout so A needs no transpose).  exp of all scores
(8.4M elem/core) is the dominant elementwise cost; it is SPLIT between the
ACT engine (exact exp) and the DVE (Schraudolph int16-bitcast affine approx,
runs at DVE 2x mode: ~0.58ns/col) so both engines land at ~58us busy:
  - Strip mapping r = 2*head + (t%2): j-tile t of head hh computes on PE row
    strip r.  Q^T is written strip-replicated directly by the Q projection
    (host passes Wq with columns [h0|h0|h1|h1]), no SBUF->SBUF copies.
    The K bias is dropped (cancels in softmax).
  - Slot structure: one slot per j-tile pair p (4 S blocks).  ACT pairs'
    blocks go through a 2-buf 2-bank "ps" PSUM tag consumed ONLY by ACT;
    Schraudolph pairs' blocks 4p+2,4p+3 go through a 1-buf "psv" tag whose
    PE fill is deferred one slot, so the saturated DVE queue can lag ~3
    slots without ever stalling the ACT exp rotation (coupling the two
    rotations was measured to cost ~10us in boundary stalls).
  - The pair's merged EA = E * A tensor_tensor (one 4D-AP op, 2x bf16)
    follows its Schraudolph in the same DVE queue -> no cross-engine wait.
  - Startup: each dma_start costs ~620ns of SERIAL sync-sequencer issue and
    per-queue rings are FIFO in posting order, so inputs are packed into 7
    fat DMAs in need-order (hT pieces are single 2KB runs per partition via
    a column-blocked host layout; all weights/biases in one "wpack" buffer;
    chunk-0 A split in two halves).  A dummy exp pulls the ~1.3us
    ACT_TABLE_LOAD into the preamble.  A few dummy matmuls warm the PE
    clock gate (kept small: a dense burst across 8 cores trips the
    chip-wide power throttle, which also makes run-to-run timing vary
    by ~10-20%).
  - phase 2 quads trail ~2 slots behind, 4 column-tiled streams into one
    PSUM tile: out_h0 (rows 0-31), out_h1 (32-63), denom_h0 (row 64),
    denom_h1 (row 96); late quads spill into the next chunk's slots 0-2.
    PSUM->SBUF output eviction runs on ACT (Copy) to keep DVE clear.
Device returns [128, N]: rows 0-31 outT_h0, 32-63 outT_h1, rows 64/96 the
softmax denominators.  Host does out = (outT / denom)^T plus the gather.

BASS_SCHR picks the per-chunk Schraudolph pairs (default 4;4;4;3 of 8 --
~23% of elements approximated, end-to-end rel err ~4.3e-3 vs 3.4e-3 exact).
"""

import math
import os

import numpy as np
import ml_dtypes

B, N, IN_DIM = 2, 2048, 256
HEADS, D = 8, 32
NCORES = 8
HPC = 2              # heads per core
NJ = N // 128        # 16 j-tiles
NCH = 4              # i-chunks
CH = N // NCH        # 512
CORES_PER_B = NCORES // B
GRP = 3              # S psum banks per exp op
NBLK = NJ * HPC      # 32 S blocks per chunk
NGRP = (NBLK + GRP - 1) // GRP   # 11 exp groups per chunk
NPAIR = NJ // 2      # 8 j-tile pairs

LAST_RESULTS = None  # BassKernelResults of the most recent kernel() call


def _build_bass():
    import concourse.bass as bass
    import concourse.mybir as mybir
    import concourse.tile as tile
    from concourse import bacc

    f32 = mybir.dt.float32
    bf16 = mybir.dt.bfloat16
    i16 = mybir.dt.int16
    AF = mybir.ActivationFunctionType

    # Schraudolph exp: bf16 bit pattern of exp(x) ~ int16(x*128*log2(e) +
    # 127*128 - c).  Piecewise-linear 2^frac approx, max rel err ~3.4%;
    # softmax normalization cancels most of it (numpy sim of the full
    # pipeline: ~1.0e-2 end-to-end at 25% of groups approximated, vs the
    # 2e-2 gate).  Offloads ACT -> DVE.
    SCH_A = 128 * 1.4426950408889634
    SCH_B = 127.0 * 128 - 7.0
    # per-chunk exp groups computed on DVE via Schraudolph instead of ACT
    # per-chunk ODD pairs whose blocks 4p+2,4p+3 are Schraudolphed on DVE
    # via a dedicated 1-buf "psv" PSUM tag (deferred fill) so the ACT exp
    # rotation never waits on the saturated DVE queue; last chunk keeps
    # pair 7 on ACT so the drain isn't DVE-gated
    _schr = os.environ.get("BASS_SCHR", "1,3,5,7;1,3,5,7;1,3,5,7;1,3,5")
    SCHRAUD = [tuple(int(x) for x in part.split(",") if x != "")
               for part in _schr.split(";")]
    # NOTE: offloading EA tensor_tensors to GPSIMD was measured
    # NET-NEGATIVE on hardware: GPSIMD streams through the DVE's second
    # SBUF port, and concurrent GPS TTs degrade DVE tensor_tensor from
    # ~1133ns to ~1430-2731ns per op (DVE lost ~23us to save 13us).

    nc = bacc.Bacc("TRN2", target_bir_lowering=False, debug=False,
                   num_devices=NCORES)

    # hT and A arrive host-relaid so each SBUF partition's bytes are one
    # contiguous DRAM run (8KB descriptors instead of 1-2KB: 16x fewer
    # descriptors -> faster SWDGE descgen and lower DMA-queue occupancy)
    hT = nc.dram_tensor("hT", [128, 2 * N], bf16, kind="ExternalInput").ap()
    Ab = nc.dram_tensor("Ab", [NCH * 128, NJ * CH], bf16,
                        kind="ExternalInput").ap()
    # all weights/biases packed in one buffer: each dma_start costs ~620ns
    # of serial sync-sequencer issue regardless of size, so one fat DMA
    # beats five thin ones.  bf16 cols: wq[s0|s1] 0:256, wk 256:384,
    # wv 384:512, bq4(f32) 512:514, bvb(f32) 514:642.
    WPK = 4 * D * 2 + HPC * D * 2 * 2 + 2 + HPC * D * 2
    wpack = nc.dram_tensor("wpack", [128, WPK], bf16,
                           kind="ExternalInput").ap()
    o = nc.dram_tensor("o", [128, N], f32, kind="ExternalOutput").ap()

    SC = 1.0 / math.sqrt(D)

    def bcast_free(ap_col, n):
        return bass.AP(tensor=ap_col.tensor, offset=ap_col.offset,
                       ap=[ap_col.ap[0], [0, n]])

    # block index for (head hh, j-tile t): strip r = 2*hh + t%2
    def blk(hh, t):
        return 4 * (t // 2) + 2 * hh + (t % 2)

    with (
        tile.TileContext(nc) as tc,
        tc.tile_pool(name="const", bufs=1) as cpool,
        tc.tile_pool(name="ps", bufs=2, space="PSUM") as pspool,
        tc.tile_pool(name="pod", bufs=2, space="PSUM") as podpool,
        tc.tile_pool(name="apool", bufs=2) as apool,
        tc.tile_pool(name="epool", bufs=2) as epool,
        tc.tile_pool(name="eapool", bufs=2) as eapool,
        tc.tile_pool(name="opool", bufs=2) as opool,
    ):
        # ---- constants / inputs into SBUF
        scratch = cpool.tile([128, CH], bf16, tag="scratch")
        nc.vector.memset(scratch, 0.0)
        ones_sb = cpool.tile([128, 1], bf16, tag="ones")
        nc.gpsimd.memset(ones_sb, 1.0)
        # dummy activation to pull the ~1.3us exp ACT_TABLE_LOAD into the
        # preamble instead of serializing before the first real exp
        warm_sb = cpool.tile([128, 1], bf16, tag="warm")
        nc.scalar.activation(warm_sb, ones_sb, AF.Exp)

        # startup DMAs in need-order (per-queue rings are FIFO in posting
        # order, so later DMAs queue behind earlier ones): Q/K-proj columns
        # of hT, the packed weights, the rest of hT half 0, the hT half-1
        # head for K-proj c1, then the chunk-0 A tile in two halves so the
        # early pairs' EA can start while the second half streams.
        # hT DRAM layout is column-blocked: [p, q(4), s(2), 512] so each
        # 512-col piece is ONE 2KB run per partition (128 fat descriptors)
        hT4 = hT.rearrange("p (q s n) -> p q s n", q=4, s=2)
        hT_half = [cpool.tile([128, 2, N // 2], bf16, tag=f"hT{c}",
                              name=f"hT{c}")
                   for c in range(2)]
        wpk_sb = cpool.tile([128, WPK], bf16, tag="wpk")
        nc.sync.dma_start(hT_half[0][:, :, 0:CH], hT4[:, 0])
        nc.sync.dma_start(wpk_sb, wpack)
        nc.sync.dma_start(hT_half[0][:, :, CH:2 * CH], hT4[:, 1])
        nc.sync.dma_start(hT_half[1][:, :, 0:CH], hT4[:, 2])

        _woff = {"q": (0, 4 * D), "k": (2 * 4 * D, HPC * D),
                 "v": (2 * 4 * D + 2 * HPC * D, HPC * D)}

        def wcols(name, s):
            off, m = _woff[name]
            return wpk_sb[:, off + s * m:off + (s + 1) * m]

        bq4_sb = wpk_sb[:, 512:514].bitcast(f32)
        bvb_sb = wpk_sb[:, 514:642].bitcast(f32)

        def hT_sb_cols(s, lo, hi):       # [lo, hi) within one half
            c = lo // (N // 2)
            assert (hi - 1) // (N // 2) == c
            return hT_half[c][:, s, lo - c * N // 2:hi - c * N // 2]

        qt4 = cpool.tile([128, N], bf16, tag="qt4")      # strips [h0|h0|h1|h1]
        kt4 = cpool.tile([128, NJ // 2 * 128], bf16, tag="kt4")
        Vt = cpool.tile([128, NJ * HPC * D], bf16, tag="vt")   # col = t*64+d

        A3 = Ab.rearrange("(c p) (t i) -> c p t i", p=128, i=CH)
        a_tiles = [None] * NCH

        def emit_a_dma(ch):
            a_t = apool.tile([128, NJ, CH], bf16, tag="a")
            nc.sync.dma_start(a_t, A3[ch])
            a_tiles[ch] = a_t

        # chunk-0 A in two halves bracketing the hT half-1 tail so the
        # K-proj c1 / late V-proj columns aren't stuck behind 2.1MB of A
        a0_t = apool.tile([128, NJ, CH], bf16, tag="a")
        nc.sync.dma_start(hT_half[1][:, :, CH:2 * CH], hT4[:, 3])
        nc.sync.dma_start(a0_t[:, 0:NJ // 2, :], A3[0][:, 0:NJ // 2])
        nc.sync.dma_start(a0_t[:, NJ // 2:, :], A3[0][:, NJ // 2:])
        a_tiles[0] = a0_t

        # ---- PE warm-up: a few dummy matmuls while input DMAs are in
        #      flight (kept small: a dense burst across all 8 cores trips
        #      the chip-wide power throttle and downclocks everything 1.2x)
        for _ in range(3):
            jt = podpool.tile([128, CH], f32, tag="od", name="junk")
            nc.tensor.matmul(jt, lhsT=scratch[:, 0:128], rhs=scratch,
                             start=True, stop=True)

        # ---- V projection helpers (all 16 tiles run inline in chunk 0)
        def emit_vproj_mms(vps, base, t0, cnt):
            for t in range(t0, t0 + cnt):
                for s in range(2):
                    nc.tensor.matmul(
                        vps[:, (t - base) * HPC * D:(t - base + 1) * HPC * D],
                        lhsT=hT_sb_cols(s, t * 128, (t + 1) * 128),
                        rhs=wcols("v", s),
                        start=(s == 0), stop=(s == 1),
                    )

        def emit_vproj_add(vps, t0, cnt=8):
            vsl = vps[:, (t0 % 8) * HPC * D:(t0 % 8 + cnt) * HPC * D]
            base = Vt[:, t0 * HPC * D:(t0 + cnt) * HPC * D]
            out_ap = bass.AP(tensor=base.tensor, offset=base.offset,
                             ap=[base.ap[0], [HPC * D, cnt], [1, HPC * D]])
            in_ap = bass.AP(tensor=vsl.tensor, offset=vsl.offset,
                            ap=[vsl.ap[0], [HPC * D, cnt], [1, HPC * D]])
            b_ap = bass.AP(tensor=bvb_sb.tensor, offset=bvb_sb.offset,
                           ap=[bvb_sb.ap[0], [0, cnt], [1, HPC * D]])
            nc.vector.tensor_add(out_ap, in_ap, b_ap)

        # ---- K projection into packed strip layout.
        #      strip r holds K^T of head r//2 for tiles t = 2*q2 + r%2.
        #      c=0 (j-tiles 0-7 -> S groups 0-4) runs on the first hT half
        #      before the loop; c=1 is emitted inside chunk 0, group 0.
        def emit_kproj(c):
            kps = pspool.tile([128, CH], f32, tag="ps", name=f"kps{c}")
            for r in range(4):
                for s in range(2):
                    base = hT_half[c][:, s, (r % 2) * 128:(r % 2) * 128 + 128]
                    rhs = bass.AP(tensor=base.tensor, offset=base.offset,
                                  ap=[base.ap[0], [256, 4], [1, 128]])
                    nc.tensor.matmul(
                        kps[32 * r:32 * r + 32, :],
                        lhsT=wcols("k", s)[:, (r // 2) * D:(r // 2 + 1) * D],
                        rhs=rhs,
                        start=(s == 0), stop=(s == 1),
                        tile_position=(0, 32 * r),
                    )
            return kps

        # bk dropped (cancels in softmax); cast on DVE so it runs in
        # parallel with the Q-proj eviction on ACT (startup critical path)
        kps0 = emit_kproj(0)
        nc.vector.tensor_copy(kt4[:, 0:CH], kps0[:, 0:CH])

        # ---- Q projection (+bias, scaled 1/sqrt(D)), strip-replicated
        #      directly: wq columns are [h0|h0|h1|h1] (host-packed).
        def emit_qproj(quarter, on_act=False):
            sl = slice(quarter * CH, (quarter + 1) * CH)
            qps = pspool.tile([128, CH], f32, tag="ps", name="qps")
            for s in range(2):
                nc.tensor.matmul(qps, lhsT=wcols("q", s),
                                 rhs=hT_sb_cols(s, quarter * CH,
                                                (quarter + 1) * CH),
                                 start=(s == 0), stop=(s == 1))
            if on_act:
                nc.scalar.activation(qt4[:, sl], qps, AF.Identity,
                                     bias=bq4_sb, scale=SC)
            else:
                nc.vector.scalar_tensor_tensor(
                    qt4[:, sl], qps, SC, bcast_free(bq4_sb, CH),
                    op0=mybir.AluOpType.mult, op1=mybir.AluOpType.add,
                )

        emit_qproj(0, on_act=True)

        # ---- main loop helpers
        def emit_amult(e_t, ea_t, a_t, p, hh):
            engine = nc.vector
            if hh is None:   # both heads of pair p in one op (4D AP)
                b0 = 4 * p * CH
                eb = e_t[:, b0:b0 + CH]
                e_ap = bass.AP(tensor=eb.tensor, offset=eb.offset,
                               ap=[eb.ap[0], [2 * CH, 2], [CH, 2], [1, CH]])
                eab = ea_t[:, b0:b0 + CH]
                ea_ap = bass.AP(tensor=eab.tensor, offset=eab.offset,
                                ap=[eab.ap[0], [2 * CH, 2], [CH, 2], [1, CH]])
                ab = a_t[:, 2 * p, :]
                a_ap = bass.AP(tensor=ab.tensor, offset=ab.offset,
                               ap=[ab.ap[0], [0, 2], [CH, 2], [1, CH]])
                engine.tensor_mul(ea_ap, e_ap, a_ap)
                return
            # blocks 4p+2hh, 4p+2hh+1 = head hh, j-tiles 2p, 2p+1: contiguous
            b0 = (4 * p + 2 * hh) * CH
            eb = e_t[:, b0:b0 + CH]
            e_ap = bass.AP(tensor=eb.tensor, offset=eb.offset,
                           ap=[eb.ap[0], [CH, 2], [1, CH]])
            eab = ea_t[:, b0:b0 + CH]
            ea_ap = bass.AP(tensor=eab.tensor, offset=eab.offset,
                            ap=[eab.ap[0], [CH, 2], [1, CH]])
            engine.tensor_mul(ea_ap, e_ap, a_t[:, 2 * p:2 * p + 2, :])

        def emit_ph2_quad(od, e_t, ea_t, p, hh, den_first=False):
            def den(tp):
                t = 2 * p + tp
                bsl = slice(blk(hh, t) * CH, (blk(hh, t) + 1) * CH)
                nc.tensor.matmul(
                    od[64 + 32 * hh:65 + 32 * hh, :],
                    lhsT=ones_sb, rhs=e_t[:, bsl],
                    start=(t == 0), stop=(t == NJ - 1),
                    tile_position=(0, 64 + 32 * hh),
                )

            def vmm(tp):
                t = 2 * p + tp
                bsl = slice(blk(hh, t) * CH, (blk(hh, t) + 1) * CH)
                nc.tensor.matmul(
                    od[32 * hh:32 * hh + 32, :],
                    lhsT=Vt[:, t * 64 + 32 * hh:t * 64 + 32 * hh + 32],
                    rhs=ea_t[:, bsl],
                    start=(t == 0), stop=(t == NJ - 1),
                    tile_position=(0, 32 * hh),
                )

            if den_first:   # dens only need E, not the DVE product
                den(0), den(1), vmm(0), vmm(1)
            else:
                vmm(0), den(0), vmm(1), den(1)

        def emit_out(od, ch):
            # PSUM->SBUF eviction on DVE: it fires at the next chunk's
            # slot ~2, exactly the window where DVE idles waiting for that
            # chunk's first exps (ACT is streaming exps then).  DMA cannot
            # read PSUM directly.
            o_sb = opool.tile([128, CH], f32, tag="o")
            nc.vector.tensor_copy(o_sb, od)
            nc.sync.dma_start(o[:, ch * CH:(ch + 1) * CH], o_sb)

        carry = None   # (od, e_t, ea_t, ch, quads) spill of previous chunk
        for ch in range(NCH):
            if ch + 1 < NCH:
                emit_a_dma(ch + 1)
            a_t = a_tiles[ch]
            e_t = epool.tile([128, NBLK * CH], bf16, tag="e")
            ea_t = eapool.tile([128, NBLK * CH], bf16, tag="ea")
            od = None

            # slot schedule: one slot per pair p.  Each slot computes the
            # pair's 4 S blocks: blocks 4p,4p+1 into a 2-bank "ps" tile and
            # 4p+2(,4p+3) into another, all consumed by ACT exp; for
            # Schraudolph pairs block 4p+3 goes to a separate 1-bank "psv"
            # tile consumed by DVE, so the two PSUM rotations never couple
            # the ACT exp stream to the saturated DVE queue.  The pair's
            # merged EA tensor_tensor follows its Schraudolph in the same
            # DVE queue; phase2 quads trail by l0 slots and spill into the
            # next chunk's slots 0-2.
            last = ch == NCH - 1
            sch_pairs = SCHRAUD[ch] if ch < len(SCHRAUD) else ()
            amult_at = {}
            epilogue_amults = []
            ph2_at = {}
            spill = []

            # od accumulation (start at t=0) requires quads of one head to
            # hit the PE queue in j-tile order: clamp to the head's running
            # max slot (append order within a slot is p-ascending).
            last_gq = [0, 0]

            def sched_quad(gq, p, hh):
                gq = max(gq, last_gq[hh])
                last_gq[hh] = gq
                if gq < NPAIR:
                    ph2_at.setdefault(gq, []).append((p, hh))
                else:
                    spill.append((p, hh))

            for p in range(NPAIR):
                schr = p in sch_pairs
                base_slot = p + 1 if schr else p
                if ch == 0:
                    # chunk-0 EA waits the A0 halves landing (~slots 2/4)
                    base_slot = max(base_slot, 2 if p < 4 else p)
                if last and p >= 6:      # tail: per-head DVE TTs, min lag
                    for hh in range(HPC):
                        amult_at.setdefault(p, []).append((p, hh))
                        sched_quad(min(p + 1, NPAIR - 1), p, hh)
                else:
                    if base_slot < NPAIR:
                        amult_at.setdefault(base_slot, []).append((p, None))
                    else:
                        epilogue_amults.append((p, None))
                    for hh in range(HPC):
                        sched_quad(base_slot + 2 + hh, p, hh)

            def emit_s(dest, b):
                q2, r = b // 4, b % 4
                nc.tensor.matmul(
                    dest,
                    lhsT=kt4[32 * r:32 * r + 32, q2 * 128:(q2 + 1) * 128],
                    rhs=qt4[32 * r:32 * r + 32, ch * CH:(ch + 1) * CH],
                    start=True, stop=True,
                    tile_position=(32 * r, 0),
                )

            def emit_schr_pair(pp):
                # 1-buf psv tile, PE-fill deferred one slot: schraud(pp-2)
                # has ~3 slots to run before this fill waits on it
                tv = pspool.tile([128, 2 * CH], f32, tag="psv", bufs=1,
                                 name="sv")
                emit_s(tv[:, 0:CH], 4 * pp + 2)
                emit_s(tv[:, CH:], 4 * pp + 3)
                nc.vector.tensor_scalar(
                    e_t[:, (4 * pp + 2) * CH:(4 * pp + 4) * CH].bitcast(i16),
                    tv, SCH_A, SCH_B,
                    op0=mybir.AluOpType.mult, op1=mybir.AluOpType.add)

            pending_schr = None
            for p in range(NPAIR):
                b0 = 4 * p
                schr = p in sch_pairs
                ta = pspool.tile([128, 2 * CH], f32, tag="ps", name="sA")
                emit_s(ta[:, 0:CH], b0)
                emit_s(ta[:, CH:], b0 + 1)
                if not schr:
                    tb = pspool.tile([128, 2 * CH], f32, tag="ps", name="sB")
                    emit_s(tb[:, 0:CH], b0 + 2)
                    emit_s(tb[:, CH:], b0 + 3)
                if pending_schr is not None:
                    emit_schr_pair(pending_schr)
                    pending_schr = None
                if schr:
                    pending_schr = p

                nc.scalar.activation(e_t[:, b0 * CH:(b0 + 2) * CH], ta,
                                     AF.Exp)
                if not schr:
                    nc.scalar.activation(
                        e_t[:, (b0 + 2) * CH:(b0 + 4) * CH], tb, AF.Exp)

                for pp, hh in amult_at.get(p, ()):
                    emit_amult(e_t, ea_t, a_t, pp, hh)

                # drain previous chunk's spilled phase2 quads: 2 per slot,
                # finishing at slot 2
                if carry is not None and p <= 2:
                    cod, ce, cea, cch, cquads = carry
                    take = cquads[:2] if p < 2 else cquads
                    for pp, hh in take:
                        emit_ph2_quad(cod, ce, cea, pp, hh)
                    cquads = cquads[len(take):]
                    if not cquads:
                        emit_out(cod, cch)
                        carry = None
                    else:
                        carry = (cod, ce, cea, cch, cquads)

                if ch == 0:
                    if p == 1:
                        # K projection second half at slot 1 (hT half 1
                        # lands ~slot 0.5; emitting at slot 0 head-of-line
                        # blocks the PE queue on the DMA); cast on ACT so
                        # the "ps" rotation's consumers stay single-engine
                        kps1 = emit_kproj(1)
                        nc.scalar.activation(kt4[:, CH:2 * CH], kps1,
                                             AF.Copy)
                    if p < 4:
                        if p == 0:
                            vps0 = podpool.tile([128, CH], f32, tag="od",
                                                name="vps0")
                        emit_vproj_mms(vps0, 0, 2 * p, 2)
                        if p == 1:
                            emit_vproj_add(vps0, 0, 4)
                        elif p == 3:
                            emit_vproj_add(vps0, 4, 4)
                    else:
                        if p == 4:
                            vps1 = podpool.tile([128, CH], f32, tag="od",
                                                name="vps1")
                        emit_vproj_mms(vps1, 8, 2 * p, 2)
                        if p == 5:
                            emit_vproj_add(vps1, 8, 4)
                        elif p == 7:
                            emit_vproj_add(vps1, 12, 4)
                    if p == 5:
                        emit_qproj(1, on_act=True)
                elif ch in (1, 2) and p == 5:
                    emit_qproj(ch + 1, on_act=True)

                for pp, hh in ph2_at.get(p, ()):
                    if od is None:
                        od = podpool.tile([128, CH], f32, tag="od")
                    emit_ph2_quad(od, e_t, ea_t, pp, hh)

            # chunk epilogue: pair-7's deferred Schraudolph + EA
            if pending_schr is not None:
                emit_schr_pair(pending_schr)
                pending_schr = None
            for pp, hh in epilogue_amults:
                emit_amult(e_t, ea_t, a_t, pp, hh)

            carry = (od, e_t, ea_t, ch, spill)

        cod, ce, cea, cch, cquads = carry
        for p, hh in cquads:
            emit_ph2_quad(cod, ce, cea, p, hh)
        emit_out(cod, cch)

    nc.finalize()
    return nc


def kernel(h, A, Wq, bq, Wk, bk, Wv, bv):
    global LAST_RESULTS
    from concourse.bass_utils import run_bass_kernel_spmd

    h = np.asarray(h, np.float32)
    A = np.asarray(A, np.float32)
    Wq = np.asarray(Wq, np.float32)
    Wk = np.asarray(Wk, np.float32)
    Wv = np.asarray(Wv, np.float32)
    bq = np.asarray(bq, np.float32)
    bv = np.asarray(bv, np.float32)

    # hT: [b, 128(p), q(4), s(2), 512] column-blocked so each 512-col DMA
    # piece is one contiguous 2KB run per partition
    hT = (h.transpose(0, 2, 1)                     # [b, IN_DIM, N]
          .reshape(B, 2, 128, N).transpose(0, 2, 1, 3)
          .reshape(B, 128, 2, 4, 512).transpose(0, 1, 3, 2, 4)
          .reshape(B, 128, 2 * N))
    hT = np.ascontiguousarray(hT).astype(ml_dtypes.bfloat16)
    # A: [b, ch*128(p), t*CH(i)]: a_t[p, t, i] = A[b, t*128+p, ch*CH+i]
    Ab = (A.reshape(B, NJ, 128, NCH, CH).transpose(0, 3, 2, 1, 4)
          .reshape(B, NCH * 128, NJ * CH))
    Ab = np.ascontiguousarray(Ab).astype(ml_dtypes.bfloat16)
    sc = np.float32(1.0 / math.sqrt(D))

    in_maps = []
    for c in range(NCORES):
        b = c // CORES_PER_B
        h0 = HPC * (c % CORES_PER_B)
        sl = slice(h0 * D, (h0 + HPC) * D)
        wq_h = [Wq[:, (h0 + k) * D:(h0 + k + 1) * D] for k in range(HPC)]
        wq_rep = np.concatenate([wq_h[0], wq_h[0], wq_h[1], wq_h[1]], axis=1)
        bq_h = [bq[(h0 + k) * D:(h0 + k + 1) * D] for k in range(HPC)]
        bq4 = np.concatenate([bq_h[0], bq_h[0], bq_h[1], bq_h[1]]) * sc
        # one packed [128, 642] bf16 buffer: wq s0|s1, wk s0|s1, wv s0|s1
        # (each [128, m] slab), then bq4/bvb as raw f32 bytes
        slabs = []
        for w in (wq_rep, Wk[:, sl], Wv[:, sl]):
            wb = w.astype(ml_dtypes.bfloat16).view(np.uint16)
            slabs += [wb[0:128, :], wb[128:256, :]]
        slabs.append(np.ascontiguousarray(
            bq4.reshape(128, 1).astype(np.float32)).view(np.uint16))
        slabs.append(np.ascontiguousarray(
            np.tile(bv[sl][None, :], (128, 1)).astype(np.float32))
            .view(np.uint16))
        wpk = np.ascontiguousarray(
            np.concatenate(slabs, axis=1)).view(ml_dtypes.bfloat16)
        in_maps.append({
            "hT": hT[b],
            "Ab": Ab[b],
            "wpack": wpk,
        })

    nc = _build_bass()
    res = run_bass_kernel_spmd(
        nc, in_maps, core_ids=list(range(NCORES)),
        trace=os.environ.get("BASS_TRACE", "0") == "1",
    )
    LAST_RESULTS = res

    out = np.empty((B, HEADS, N, D), np.float32)
    for c in range(NCORES):
        b = c // CORES_PER_B
        h0 = HPC * (c % CORES_PER_B)
        oo = res.results[c]["o"]                  # [128, N] f32
        for hh in range(HPC):
            num = oo[hh * D:(hh + 1) * D, :]      # [32, N] unnormalized out^T
            den = oo[64 + 32 * hh, :]             # [N]
            out[b, h0 + hh] = (num / den[None, :]).T
    return out



# revision 44
# speedup vs baseline: 1.0103x; 1.0103x over previous
"""Adjacency-aware multi-head attention on 8 trn2 NeuronCores.

Math (per b, head k):
  Q = h[b] @ Wq[:, k] + bq[k]           [N, D]
  S[i, j] = (Q_i . K_j) / sqrt(D)
  P[j, i] = exp(S[i, j]) / sum_j exp(S[i, j])      (softmax over keys j)
  out[i, d] = sum_j P[j, i] * A[b, j, i] * V[j, d]

The K bias cancels: it adds g[i] = Q_i . bk to every score of query i,
and softmax over j is invariant to per-i shifts -> bk is dropped.

Sharding: 16 (b, head) pairs over 8 cores, 2 heads of the SAME b per core so
the A[b] stream is shared by both heads.

Device dataflow ([j, i] layout so A needs no transpose).  exp of all scores
(8.4M elem/core) is the dominant elementwise cost; it is SPLIT between the
ACT engine (exact exp) and the DVE (Schraudolph int16-bitcast affine approx,
runs at DVE 2x mode: ~0.58ns/col) so both engines land at ~58us busy:
  - Strip mapping r = 2*head + (t%2): j-tile t of head hh computes on PE row
    strip r.  Q^T is written strip-replicated directly by the Q projection
    (host passes Wq with columns [h0|h0|h1|h1]), no SBUF->SBUF copies.
    The K bias is dropped (cancels in softmax).
  - Slot structure: one slot per j-tile pair p (4 S blocks).  ACT pairs'
    blocks go through a 2-buf 2-bank "ps" PSUM tag consumed ONLY by ACT;
    Schraudolph pairs' blocks 4p+2,4p+3 go through a 1-buf "psv" tag whose
    PE fill is deferred one slot, so the saturated DVE queue can lag ~3
    slots without ever stalling the ACT exp rotation (coupling the two
    rotations was measured to cost ~10us in boundary stalls).
  - The pair's merged EA = E * A tensor_tensor (one 4D-AP op, 2x bf16)
    follows its Schraudolph in the same DVE queue -> no cross-engine wait.
  - Startup: each dma_start costs ~620ns of SERIAL sync-sequencer issue and
    per-queue rings are FIFO in posting order, so inputs are packed into 7
    fat DMAs in need-order (hT pieces are single 2KB runs per partition via
    a column-blocked host layout; all weights/biases in one "wpack" buffer;
    chunk-0 A split in two halves).  A dummy exp pulls the ~1.3us
    ACT_TABLE_LOAD into the preamble.  A few dummy matmuls warm the PE
    clock gate (kept small: a dense burst across 8 cores trips the
    chip-wide power throttle, which also makes run-to-run timing vary
    by ~10-20%).
  - phase 2 quads trail ~2 slots behind, 4 column-tiled streams into one
    PSUM tile: out_h0 (rows 0-31), out_h1 (32-63), denom_h0 (row 64),
    denom_h1 (row 96); late quads spill into the next chunk's slots 0-2.
    PSUM->SBUF output eviction runs on ACT (Copy) to keep DVE clear.
Device returns [128, N]: rows 0-31 outT_h0, 32-63 outT_h1, rows 64/96 the
softmax denominators.  Host does out = (outT / denom)^T plus the gather.

BASS_SCHR picks the per-chunk Schraudolph pairs (default 4;4;4;3 of 8 --
~23% of elements approximated, end-to-end rel err ~4.3e-3 vs 3.4e-3 exact).
"""

import math
import os

import numpy as np
import ml_dtypes

B, N, IN_DIM = 2, 2048, 256
HEADS, D = 8, 32
NCORES = 8
HPC = 2              # heads per core
NJ = N // 128        # 16 j-tiles
NCH = 4              # i-chunks
CH = N // NCH        # 512
CORES_PER_B = NCORES // B
GRP = 3              # S psum banks per exp op
NBLK = NJ * HPC      # 32 S blocks per chunk
NGRP = (NBLK + GRP - 1) // GRP   # 11 exp groups per chunk
NPAIR = NJ // 2      # 8 j-tile pairs

LAST_RESULTS = None  # BassKernelResults of the most recent kernel() call


def _build_bass():
    import concourse.bass as bass
    import concourse.mybir as mybir
    import concourse.tile as tile
    from concourse import bacc

    f32 = mybir.dt.float32
    bf16 = mybir.dt.bfloat16
    i16 = mybir.dt.int16
    AF = mybir.ActivationFunctionType

    # Schraudolph exp: bf16 bit pattern of exp(x) ~ int16(x*128*log2(e) +
    # 127*128 - c).  Piecewise-linear 2^frac approx, max rel err ~3.4%;
    # softmax normalization cancels most of it (numpy sim of the full
    # pipeline: ~1.0e-2 end-to-end at 25% of groups approximated, vs the
    # 2e-2 gate).  Offloads ACT -> DVE.
    SCH_A = 128 * 1.4426950408889634
    SCH_B = 127.0 * 128 - 7.0
    # per-chunk exp groups computed on DVE via Schraudolph instead of ACT
    # per-chunk ODD pairs whose blocks 4p+2,4p+3 are Schraudolphed on DVE
    # via a dedicated 1-buf "psv" PSUM tag (deferred fill) so the ACT exp
    # rotation never waits on the saturated DVE queue; last chunk keeps
    # pair 7 on ACT so the drain isn't DVE-gated
    _schr = os.environ.get("BASS_SCHR", "1,3,5,7;1,3,5,7;1,3,5,7;1,3,5")
    SCHRAUD = [tuple(int(x) for x in part.split(",") if x != "")
               for part in _schr.split(";")]
    # NOTE: offloading EA tensor_tensors to GPSIMD was measured
    # NET-NEGATIVE on hardware: GPSIMD streams through the DVE's second
    # SBUF port, and concurrent GPS TTs degrade DVE tensor_tensor from
    # ~1133ns to ~1430-2731ns per op (DVE lost ~23us to save 13us).

    nc = bacc.Bacc("TRN2", target_bir_lowering=False, debug=False,
                   num_devices=NCORES)

    # hT and A arrive host-relaid so each SBUF partition's bytes are one
    # contiguous DRAM run (8KB descriptors instead of 1-2KB: 16x fewer
    # descriptors -> faster SWDGE descgen and lower DMA-queue occupancy)
    hT = nc.dram_tensor("hT", [128, 2 * N], bf16, kind="ExternalInput").ap()
    Ab = nc.dram_tensor("Ab", [NCH * 128, NJ * CH], bf16,
                        kind="ExternalInput").ap()
    # all weights/biases packed in one buffer: each dma_start costs ~620ns
    # of serial sync-sequencer issue regardless of size, so one fat DMA
    # beats five thin ones.  bf16 cols: wq[s0|s1] 0:256, wk 256:384,
    # wv 384:512, bq4(f32) 512:514, bvb(f32) 514:642.
    WPK = 4 * D * 2 + HPC * D * 2 * 2 + 2 + HPC * D * 2
    wpack = nc.dram_tensor("wpack", [128, WPK], bf16,
                           kind="ExternalInput").ap()
    o = nc.dram_tensor("o", [128, N], f32, kind="ExternalOutput").ap()

    SC = 1.0 / math.sqrt(D)

    def bcast_free(ap_col, n):
        return bass.AP(tensor=ap_col.tensor, offset=ap_col.offset,
                       ap=[ap_col.ap[0], [0, n]])

    # block index for (head hh, j-tile t): strip r = 2*hh + t%2
    def blk(hh, t):
        return 4 * (t // 2) + 2 * hh + (t % 2)

    with (
        tile.TileContext(nc) as tc,
        tc.tile_pool(name="const", bufs=1) as cpool,
        tc.tile_pool(name="ps", bufs=2, space="PSUM") as pspool,
        tc.tile_pool(name="pod", bufs=2, space="PSUM") as podpool,
        tc.tile_pool(name="apool", bufs=2) as apool,
        tc.tile_pool(name="epool", bufs=2) as epool,
        tc.tile_pool(name="eapool", bufs=2) as eapool,
        tc.tile_pool(name="opool", bufs=2) as opool,
    ):
        # ---- constants / inputs into SBUF
        scratch = cpool.tile([128, CH], bf16, tag="scratch")
        nc.vector.memset(scratch, 0.0)
        ones_sb = cpool.tile([128, 1], bf16, tag="ones")
        nc.gpsimd.memset(ones_sb, 1.0)
        # dummy activation to pull the ~1.3us exp ACT_TABLE_LOAD into the
        # preamble instead of serializing before the first real exp
        warm_sb = cpool.tile([128, 1], bf16, tag="warm")
        nc.scalar.activation(warm_sb, ones_sb, AF.Exp)

        # startup DMAs in need-order (per-queue rings are FIFO in posting
        # order, so later DMAs queue behind earlier ones): Q/K-proj columns
        # of hT, the packed weights, the rest of hT half 0, the hT half-1
        # head for K-proj c1, then the chunk-0 A tile in two halves so the
        # early pairs' EA can start while the second half streams.
        # hT DRAM layout is column-blocked: [p, q(4), s(2), 512] so each
        # 512-col piece is ONE 2KB run per partition (128 fat descriptors)
        hT4 = hT.rearrange("p (q s n) -> p q s n", q=4, s=2)
        hT_half = [cpool.tile([128, 2, N // 2], bf16, tag=f"hT{c}",
                              name=f"hT{c}")
                   for c in range(2)]
        wpk_sb = cpool.tile([128, WPK], bf16, tag="wpk")
        nc.sync.dma_start(hT_half[0][:, :, 0:CH], hT4[:, 0])
        nc.sync.dma_start(wpk_sb, wpack)
        nc.sync.dma_start(hT_half[0][:, :, CH:2 * CH], hT4[:, 1])
        nc.sync.dma_start(hT_half[1][:, :, 0:CH], hT4[:, 2])

        _woff = {"q": (0, 4 * D), "k": (2 * 4 * D, HPC * D),
                 "v": (2 * 4 * D + 2 * HPC * D, HPC * D)}

        def wcols(name, s):
            off, m = _woff[name]
            return wpk_sb[:, off + s * m:off + (s + 1) * m]

        bq4_sb = wpk_sb[:, 512:514].bitcast(f32)
        bvb_sb = wpk_sb[:, 514:642].bitcast(f32)

        def hT_sb_cols(s, lo, hi):       # [lo, hi) within one half
            c = lo // (N // 2)
            assert (hi - 1) // (N // 2) == c
            return hT_half[c][:, s, lo - c * N // 2:hi - c * N // 2]

        qt4 = cpool.tile([128, N], bf16, tag="qt4")      # strips [h0|h0|h1|h1]
        kt4 = cpool.tile([128, NJ // 2 * 128], bf16, tag="kt4")
        Vt = cpool.tile([128, NJ * HPC * D], bf16, tag="vt")   # col = t*64+d

        A3 = Ab.rearrange("(c p) (t i) -> c p t i", p=128, i=CH)
        a_tiles = [None] * NCH

        def emit_a_dma(ch):
            a_t = apool.tile([128, NJ, CH], bf16, tag="a")
            nc.sync.dma_start(a_t, A3[ch])
            a_tiles[ch] = a_t

        # chunk-0 A in two halves bracketing the hT half-1 tail so the
        # K-proj c1 / late V-proj columns aren't stuck behind 2.1MB of A
        a0_t = apool.tile([128, NJ, CH], bf16, tag="a")
        nc.sync.dma_start(hT_half[1][:, :, CH:2 * CH], hT4[:, 3])
        nc.sync.dma_start(a0_t[:, 0:NJ // 2, :], A3[0][:, 0:NJ // 2])
        nc.sync.dma_start(a0_t[:, NJ // 2:, :], A3[0][:, NJ // 2:])
        a_tiles[0] = a0_t

        # ---- PE warm-up: a few dummy matmuls while input DMAs are in
        #      flight (kept small: a dense burst across all 8 cores trips
        #      the chip-wide power throttle and downclocks everything 1.2x)
        for _ in range(3):
            jt = podpool.tile([128, CH], f32, tag="od", name="junk")
            nc.tensor.matmul(jt, lhsT=scratch[:, 0:128], rhs=scratch,
                             start=True, stop=True)

        # ---- V projection helpers (all 16 tiles run inline in chunk 0)
        def emit_vproj_mms(vps, base, t0, cnt):
            for t in range(t0, t0 + cnt):
                for s in range(2):
                    nc.tensor.matmul(
                        vps[:, (t - base) * HPC * D:(t - base + 1) * HPC * D],
                        lhsT=hT_sb_cols(s, t * 128, (t + 1) * 128),
                        rhs=wcols("v", s),
                        start=(s == 0), stop=(s == 1),
                    )

        def emit_vproj_add(vps, t0, cnt=8):
            vsl = vps[:, (t0 % 8) * HPC * D:(t0 % 8 + cnt) * HPC * D]
            base = Vt[:, t0 * HPC * D:(t0 + cnt) * HPC * D]
            out_ap = bass.AP(tensor=base.tensor, offset=base.offset,
                             ap=[base.ap[0], [HPC * D, cnt], [1, HPC * D]])
            in_ap = bass.AP(tensor=vsl.tensor, offset=vsl.offset,
                            ap=[vsl.ap[0], [HPC * D, cnt], [1, HPC * D]])
            b_ap = bass.AP(tensor=bvb_sb.tensor, offset=bvb_sb.offset,
                           ap=[bvb_sb.ap[0], [0, cnt], [1, HPC * D]])
            nc.vector.tensor_add(out_ap, in_ap, b_ap)

        # ---- K projection into packed strip layout.
        #      strip r holds K^T of head r//2 for tiles t = 2*q2 + r%2.
        #      c=0 (j-tiles 0-7 -> S groups 0-4) runs on the first hT half
        #      before the loop; c=1 is emitted inside chunk 0, group 0.
        def emit_kproj(c):
            kps = pspool.tile([128, CH], f32, tag="ps", name=f"kps{c}")
            for r in range(4):
                for s in range(2):
                    base = hT_half[c][:, s, (r % 2) * 128:(r % 2) * 128 + 128]
                    rhs = bass.AP(tensor=base.tensor, offset=base.offset,
                                  ap=[base.ap[0], [256, 4], [1, 128]])
                    nc.tensor.matmul(
                        kps[32 * r:32 * r + 32, :],
                        lhsT=wcols("k", s)[:, (r // 2) * D:(r // 2 + 1) * D],
                        rhs=rhs,
                        start=(s == 0), stop=(s == 1),
                        tile_position=(0, 32 * r),
                    )
            return kps

        # bk dropped (cancels in softmax); cast on DVE so it runs in
        # parallel with the Q-proj eviction on ACT (startup critical path)
        kps0 = emit_kproj(0)
        nc.vector.tensor_copy(kt4[:, 0:CH], kps0[:, 0:CH])

        # ---- Q projection (+bias, scaled 1/sqrt(D)), strip-replicated
        #      directly: wq columns are [h0|h0|h1|h1] (host-packed).
        def emit_qproj(quarter, on_act=False):
            sl = slice(quarter * CH, (quarter + 1) * CH)
            qps = pspool.tile([128, CH], f32, tag="ps", name="qps")
            for s in range(2):
                nc.tensor.matmul(qps, lhsT=wcols("q", s),
                                 rhs=hT_sb_cols(s, quarter * CH,
                                                (quarter + 1) * CH),
                                 start=(s == 0), stop=(s == 1))
            if on_act:
                nc.scalar.activation(qt4[:, sl], qps, AF.Identity,
                                     bias=bq4_sb, scale=SC)
            else:
                nc.vector.scalar_tensor_tensor(
                    qt4[:, sl], qps, SC, bcast_free(bq4_sb, CH),
                    op0=mybir.AluOpType.mult, op1=mybir.AluOpType.add,
                )

        emit_qproj(0, on_act=True)

        # ---- main loop helpers
        def emit_amult(e_t, ea_t, a_t, p, hh):
            engine = nc.vector
            if hh is None:   # both heads of pair p in one op (4D AP)
                b0 = 4 * p * CH
                eb = e_t[:, b0:b0 + CH]
                e_ap = bass.AP(tensor=eb.tensor, offset=eb.offset,
                               ap=[eb.ap[0], [2 * CH, 2], [CH, 2], [1, CH]])
                eab = ea_t[:, b0:b0 + CH]
                ea_ap = bass.AP(tensor=eab.tensor, offset=eab.offset,
                                ap=[eab.ap[0], [2 * CH, 2], [CH, 2], [1, CH]])
                ab = a_t[:, 2 * p, :]
                a_ap = bass.AP(tensor=ab.tensor, offset=ab.offset,
                               ap=[ab.ap[0], [0, 2], [CH, 2], [1, CH]])
                engine.tensor_mul(ea_ap, e_ap, a_ap)
                return
            # blocks 4p+2hh, 4p+2hh+1 = head hh, j-tiles 2p, 2p+1: contiguous
            b0 = (4 * p + 2 * hh) * CH
            eb = e_t[:, b0:b0 + CH]
            e_ap = bass.AP(tensor=eb.tensor, offset=eb.offset,
                           ap=[eb.ap[0], [CH, 2], [1, CH]])
            eab = ea_t[:, b0:b0 + CH]
            ea_ap = bass.AP(tensor=eab.tensor, offset=eab.offset,
                            ap=[eab.ap[0], [CH, 2], [1, CH]])
            engine.tensor_mul(ea_ap, e_ap, a_t[:, 2 * p:2 * p + 2, :])

        def emit_ph2_quad(od, e_t, ea_t, p, hh, den_first=False):
            # NOTE: splitting these 128-contraction matmuls into 4 x 32-row
            # bands at tile positions (32r, col) -- to overlap like the S
            # strips do -- fails at runtime (INTERNAL error), so the
            # full-contraction form stays.
            def den(tp):
                t = 2 * p + tp
                bsl = slice(blk(hh, t) * CH, (blk(hh, t) + 1) * CH)
                nc.tensor.matmul(
                    od[64 + 32 * hh:65 + 32 * hh, :],
                    lhsT=ones_sb, rhs=e_t[:, bsl],
                    start=(t == 0), stop=(t == NJ - 1),
                    tile_position=(0, 64 + 32 * hh),
                )

            def vmm(tp):
                t = 2 * p + tp
                bsl = slice(blk(hh, t) * CH, (blk(hh, t) + 1) * CH)
                nc.tensor.matmul(
                    od[32 * hh:32 * hh + 32, :],
                    lhsT=Vt[:, t * 64 + 32 * hh:t * 64 + 32 * hh + 32],
                    rhs=ea_t[:, bsl],
                    start=(t == 0), stop=(t == NJ - 1),
                    tile_position=(0, 32 * hh),
                )

            if den_first:   # dens only need E, not the DVE product
                den(0), den(1), vmm(0), vmm(1)
            else:
                vmm(0), den(0), vmm(1), den(1)

        def emit_out(od, ch):
            # PSUM->SBUF eviction on ACT (measured better than DVE here);
            # DMA cannot read PSUM directly.
            o_sb = opool.tile([128, CH], f32, tag="o")
            nc.scalar.activation(o_sb, od, AF.Copy)
            nc.sync.dma_start(o[:, ch * CH:(ch + 1) * CH], o_sb)

        carry = None   # (od, e_t, ea_t, ch, quads) spill of previous chunk
        for ch in range(NCH):
            if ch + 1 < NCH:
                emit_a_dma(ch + 1)
            a_t = a_tiles[ch]
            e_t = epool.tile([128, NBLK * CH], bf16, tag="e")
            ea_t = eapool.tile([128, NBLK * CH], bf16, tag="ea")
            od = None

            # slot schedule: one slot per pair p.  Each slot computes the
            # pair's 4 S blocks: blocks 4p,4p+1 into a 2-bank "ps" tile and
            # 4p+2(,4p+3) into another, all consumed by ACT exp; for
            # Schraudolph pairs block 4p+3 goes to a separate 1-bank "psv"
            # tile consumed by DVE, so the two PSUM rotations never couple
            # the ACT exp stream to the saturated DVE queue.  The pair's
            # merged EA tensor_tensor follows its Schraudolph in the same
            # DVE queue; phase2 quads trail by l0 slots and spill into the
            # next chunk's slots 0-2.
            last = ch == NCH - 1
            sch_pairs = SCHRAUD[ch] if ch < len(SCHRAUD) else ()
            amult_at = {}
            epilogue_amults = []
            ph2_at = {}
            spill = []

            # od accumulation (start at t=0) requires quads of one head to
            # hit the PE queue in j-tile order: clamp to the head's running
            # max slot (append order within a slot is p-ascending).
            last_gq = [0, 0]

            def sched_quad(gq, p, hh):
                gq = max(gq, last_gq[hh])
                last_gq[hh] = gq
                if gq < NPAIR:
                    ph2_at.setdefault(gq, []).append((p, hh))
                else:
                    spill.append((p, hh))

            for p in range(NPAIR):
                schr = p in sch_pairs
                base_slot = p + 1 if schr else p
                if ch == 0:
                    # chunk-0 EA waits the A0 halves landing (~slots 2/4)
                    base_slot = max(base_slot, 2 if p < 4 else p)
                if last and p >= 6:      # tail: per-head DVE TTs, min lag
                    for hh in range(HPC):
                        amult_at.setdefault(p, []).append((p, hh))
                        sched_quad(min(p + 1, NPAIR - 1), p, hh)
                else:
                    if base_slot < NPAIR:
                        amult_at.setdefault(base_slot, []).append((p, None))
                    else:
                        epilogue_amults.append((p, None))
                    for hh in range(HPC):
                        sched_quad(base_slot + 2 + hh, p, hh)

            def emit_s(dest, b):
                q2, r = b // 4, b % 4
                nc.tensor.matmul(
                    dest,
                    lhsT=kt4[32 * r:32 * r + 32, q2 * 128:(q2 + 1) * 128],
                    rhs=qt4[32 * r:32 * r + 32, ch * CH:(ch + 1) * CH],
                    start=True, stop=True,
                    tile_position=(32 * r, 0),
                )

            def emit_schr_pair(pp):
                # 1-buf psv tile, PE-fill deferred one slot: schraud(pp-2)
                # has ~3 slots to run before this fill waits on it
                tv = pspool.tile([128, 2 * CH], f32, tag="psv", bufs=1,
                                 name="sv")
                emit_s(tv[:, 0:CH], 4 * pp + 2)
                emit_s(tv[:, CH:], 4 * pp + 3)
                nc.vector.tensor_scalar(
                    e_t[:, (4 * pp + 2) * CH:(4 * pp + 4) * CH].bitcast(i16),
                    tv, SCH_A, SCH_B,
                    op0=mybir.AluOpType.mult, op1=mybir.AluOpType.add)

            pending_schr = None
            for p in range(NPAIR):
                b0 = 4 * p
                schr = p in sch_pairs
                ta = pspool.tile([128, 2 * CH], f32, tag="ps", name="sA")
                emit_s(ta[:, 0:CH], b0)
                emit_s(ta[:, CH:], b0 + 1)
                if not schr:
                    tb = pspool.tile([128, 2 * CH], f32, tag="ps", name="sB")
                    emit_s(tb[:, 0:CH], b0 + 2)
                    emit_s(tb[:, CH:], b0 + 3)
                if pending_schr is not None:
                    emit_schr_pair(pending_schr)
                    pending_schr = None
                if schr:
                    pending_schr = p

                nc.scalar.activation(e_t[:, b0 * CH:(b0 + 2) * CH], ta,
                                     AF.Exp)
                if not schr:
                    nc.scalar.activation(
                        e_t[:, (b0 + 2) * CH:(b0 + 4) * CH], tb, AF.Exp)

                for pp, hh in amult_at.get(p, ()):
                    emit_amult(e_t, ea_t, a_t, pp, hh)

                # drain previous chunk's spilled phase2 quads: 2 per slot,
                # finishing at slot 2
                if carry is not None and p <= 2:
                    cod, ce, cea, cch, cquads = carry
                    take = cquads[:2] if p < 2 else cquads
                    for pp, hh in take:
                        emit_ph2_quad(cod, ce, cea, pp, hh)
                    cquads = cquads[len(take):]
                    if not cquads:
                        emit_out(cod, cch)
                        carry = None
                    else:
                        carry = (cod, ce, cea, cch, cquads)

                if ch == 0:
                    if p == 1:
                        # K projection second half at slot 1 (hT half 1
                        # lands ~slot 0.5; emitting at slot 0 head-of-line
                        # blocks the PE queue on the DMA); cast on ACT so
                        # the "ps" rotation's consumers stay single-engine
                        kps1 = emit_kproj(1)
                        nc.scalar.activation(kt4[:, CH:2 * CH], kps1,
                                             AF.Copy)
                    if p < 4:
                        if p == 0:
                            vps0 = podpool.tile([128, CH], f32, tag="od",
                                                name="vps0")
                        emit_vproj_mms(vps0, 0, 2 * p, 2)
                        if p == 1:
                            emit_vproj_add(vps0, 0, 4)
                        elif p == 3:
                            emit_vproj_add(vps0, 4, 4)
                    else:
                        if p == 4:
                            vps1 = podpool.tile([128, CH], f32, tag="od",
                                                name="vps1")
                        emit_vproj_mms(vps1, 8, 2 * p, 2)
                        if p == 5:
                            emit_vproj_add(vps1, 8, 4)
                        elif p == 7:
                            emit_vproj_add(vps1, 12, 4)
                    if p == 5:
                        emit_qproj(1, on_act=True)
                elif ch in (1, 2) and p == 5:
                    emit_qproj(ch + 1, on_act=True)

                for pp, hh in ph2_at.get(p, ()):
                    if od is None:
                        od = podpool.tile([128, CH], f32, tag="od")
                    emit_ph2_quad(od, e_t, ea_t, pp, hh)

            # chunk epilogue: pair-7's deferred Schraudolph + EA
            if pending_schr is not None:
                emit_schr_pair(pending_schr)
                pending_schr = None
            for pp, hh in epilogue_amults:
                emit_amult(e_t, ea_t, a_t, pp, hh)

            carry = (od, e_t, ea_t, ch, spill)

        cod, ce, cea, cch, cquads = carry
        for p, hh in cquads:
            emit_ph2_quad(cod, ce, cea, p, hh)
        emit_out(cod, cch)

    nc.finalize()
    return nc


def kernel(h, A, Wq, bq, Wk, bk, Wv, bv):
    global LAST_RESULTS
    from concourse.bass_utils import run_bass_kernel_spmd

    h = np.asarray(h, np.float32)
    A = np.asarray(A, np.float32)
    Wq = np.asarray(Wq, np.float32)
    Wk = np.asarray(Wk, np.float32)
    Wv = np.asarray(Wv, np.float32)
    bq = np.asarray(bq, np.float32)
    bv = np.asarray(bv, np.float32)

    # hT: [b, 128(p), q(4), s(2), 512] column-blocked so each 512-col DMA
    # piece is one contiguous 2KB run per partition
    hT = (h.transpose(0, 2, 1)                     # [b, IN_DIM, N]
          .reshape(B, 2, 128, N).transpose(0, 2, 1, 3)
          .reshape(B, 128, 2, 4, 512).transpose(0, 1, 3, 2, 4)
          .reshape(B, 128, 2 * N))
    hT = np.ascontiguousarray(hT).astype(ml_dtypes.bfloat16)
    # A: [b, ch*128(p), t*CH(i)]: a_t[p, t, i] = A[b, t*128+p, ch*CH+i]
    Ab = (A.reshape(B, NJ, 128, NCH, CH).transpose(0, 3, 2, 1, 4)
          .reshape(B, NCH * 128, NJ * CH))
    Ab = np.ascontiguousarray(Ab).astype(ml_dtypes.bfloat16)
    sc = np.float32(1.0 / math.sqrt(D))

    in_maps = []
    for c in range(NCORES):
        b = c // CORES_PER_B
        h0 = HPC * (c % CORES_PER_B)
        sl = slice(h0 * D, (h0 + HPC) * D)
        wq_h = [Wq[:, (h0 + k) * D:(h0 + k + 1) * D] for k in range(HPC)]
        wq_rep = np.concatenate([wq_h[0], wq_h[0], wq_h[1], wq_h[1]], axis=1)
        bq_h = [bq[(h0 + k) * D:(h0 + k + 1) * D] for k in range(HPC)]
        bq4 = np.concatenate([bq_h[0], bq_h[0], bq_h[1], bq_h[1]]) * sc
        # one packed [128, 642] bf16 buffer: wq s0|s1, wk s0|s1, wv s0|s1
        # (each [128, m] slab), then bq4/bvb as raw f32 bytes
        slabs = []
        for w in (wq_rep, Wk[:, sl], Wv[:, sl]):
            wb = w.astype(ml_dtypes.bfloat16).view(np.uint16)
            slabs += [wb[0:128, :], wb[128:256, :]]
        slabs.append(np.ascontiguousarray(
            bq4.reshape(128, 1).astype(np.float32)).view(np.uint16))
        slabs.append(np.ascontiguousarray(
            np.tile(bv[sl][None, :], (128, 1)).astype(np.float32))
            .view(np.uint16))
        wpk = np.ascontiguousarray(
            np.concatenate(slabs, axis=1)).view(ml_dtypes.bfloat16)
        in_maps.append({
            "hT": hT[b],
            "Ab": Ab[b],
            "wpack": wpk,
        })

    nc = _build_bass()
    res = run_bass_kernel_spmd(
        nc, in_maps, core_ids=list(range(NCORES)),
        trace=os.environ.get("BASS_TRACE", "0") == "1",
    )
    LAST_RESULTS = res

    out = np.empty((B, HEADS, N, D), np.float32)
    for c in range(NCORES):
        b = c // CORES_PER_B
        h0 = HPC * (c % CORES_PER_B)
        oo = res.results[c]["o"]                  # [128, N] f32
        for hh in range(HPC):
            num = oo[hh * D:(hh + 1) * D, :]      # [32, N] unnormalized out^T
            den = oo[64 + 32 * hh, :]             # [N]
            out[b, h0 + hh] = (num / den[None, :]).T
    return out



# revision 45
# speedup vs baseline: 1.0365x; 1.0259x over previous
"""Adjacency-aware multi-head attention on 8 trn2 NeuronCores.

Math (per b, head k):
  Q = h[b] @ Wq[:, k] + bq[k]           [N, D]
  S[i, j] = (Q_i . K_j) / sqrt(D)
  P[j, i] = exp(S[i, j]) / sum_j exp(S[i, j])      (softmax over keys j)
  out[i, d] = sum_j P[j, i] * A[b, j, i] * V[j, d]

The K bias cancels: it adds g[i] = Q_i . bk to every score of query i,
and softmax over j is invariant to per-i shifts -> bk is dropped.

Sharding: 16 (b, head) pairs over 8 cores, 2 heads of the SAME b per core so
the A[b] stream is shared by both heads.

Device dataflow ([j, i] layout so A needs no transpose).  exp of all scores
(8.4M elem/core) is the dominant elementwise cost; it is SPLIT between the
ACT engine (exact exp) and the DVE (Schraudolph int16-bitcast affine approx,
runs at DVE 2x mode: ~0.58ns/col) so both engines land at ~58us busy:
  - Strip mapping r = 2*head + (t%2): j-tile t of head hh computes on PE row
    strip r.  Q^T is written strip-replicated directly by the Q projection
    (host passes Wq with columns [h0|h0|h1|h1]), no SBUF->SBUF copies.
    The K bias is dropped (cancels in softmax).
  - Slot structure: one slot per j-tile pair p (4 S blocks).  ACT pairs'
    blocks go through a 2-buf 2-bank "ps" PSUM tag consumed ONLY by ACT;
    Schraudolph pairs' blocks 4p+2,4p+3 go through a 1-buf "psv" tag whose
    PE fill is deferred one slot, so the saturated DVE queue can lag ~3
    slots without ever stalling the ACT exp rotation (coupling the two
    rotations was measured to cost ~10us in boundary stalls).
  - The pair's merged EA = E * A tensor_tensor (one 4D-AP op, 2x bf16)
    follows its Schraudolph in the same DVE queue -> no cross-engine wait.
  - Startup: each dma_start costs ~620ns of SERIAL sync-sequencer issue and
    per-queue rings are FIFO in posting order, so inputs are packed into 7
    fat DMAs in need-order (hT pieces are single 2KB runs per partition via
    a column-blocked host layout; all weights/biases in one "wpack" buffer;
    chunk-0 A split in two halves).  A dummy exp pulls the ~1.3us
    ACT_TABLE_LOAD into the preamble.  A few dummy matmuls warm the PE
    clock gate (kept small: a dense burst across 8 cores trips the
    chip-wide power throttle, which also makes run-to-run timing vary
    by ~10-20%).
  - phase 2 quads trail ~2 slots behind, 4 column-tiled streams into one
    PSUM tile: out_h0 (rows 0-31), out_h1 (32-63), denom_h0 (row 64),
    denom_h1 (row 96); late quads spill into the next chunk's slots 0-2.
    PSUM->SBUF output eviction runs on ACT (Copy) to keep DVE clear.
Device returns [128, N]: rows 0-31 outT_h0, 32-63 outT_h1, rows 64/96 the
softmax denominators.  Host does out = (outT / denom)^T plus the gather.

BASS_SCHR picks the per-chunk Schraudolph pairs (default 4;4;4;3 of 8 --
~23% of elements approximated, end-to-end rel err ~4.3e-3 vs 3.4e-3 exact).
"""

import math
import os

import numpy as np
import ml_dtypes

B, N, IN_DIM = 2, 2048, 256
HEADS, D = 8, 32
NCORES = 8
HPC = 2              # heads per core
NJ = N // 128        # 16 j-tiles
NCH = 4              # i-chunks
CH = N // NCH        # 512
CORES_PER_B = NCORES // B
GRP = 3              # S psum banks per exp op
NBLK = NJ * HPC      # 32 S blocks per chunk
NGRP = (NBLK + GRP - 1) // GRP   # 11 exp groups per chunk
NPAIR = NJ // 2      # 8 j-tile pairs

LAST_RESULTS = None  # BassKernelResults of the most recent kernel() call


def _build_bass():
    import concourse.bass as bass
    import concourse.mybir as mybir
    import concourse.tile as tile
    from concourse import bacc

    f32 = mybir.dt.float32
    bf16 = mybir.dt.bfloat16
    i16 = mybir.dt.int16
    AF = mybir.ActivationFunctionType

    # Schraudolph exp: bf16 bit pattern of exp(x) ~ int16(x*128*log2(e) +
    # 127*128 - c).  Piecewise-linear 2^frac approx, max rel err ~3.4%;
    # softmax normalization cancels most of it (numpy sim of the full
    # pipeline: ~1.0e-2 end-to-end at 25% of groups approximated, vs the
    # 2e-2 gate).  Offloads ACT -> DVE.
    SCH_A = 128 * 1.4426950408889634
    SCH_B = 127.0 * 128 - 7.0
    # per-chunk exp groups computed on DVE via Schraudolph instead of ACT
    # per-chunk ODD pairs whose blocks 4p+2,4p+3 are Schraudolphed on DVE
    # via a dedicated 1-buf "psv" PSUM tag (deferred fill) so the ACT exp
    # rotation never waits on the saturated DVE queue; last chunk keeps
    # pair 7 on ACT so the drain isn't DVE-gated
    _schr = os.environ.get("BASS_SCHR", "1,3,5,7;1,3,5,7;1,3,5,7;1,3,5")
    SCHRAUD = [tuple(int(x) for x in part.split(",") if x != "")
               for part in _schr.split(";")]
    # NOTE: offloading EA tensor_tensors to GPSIMD was measured
    # NET-NEGATIVE on hardware: GPSIMD streams through the DVE's second
    # SBUF port, and concurrent GPS TTs degrade DVE tensor_tensor from
    # ~1133ns to ~1430-2731ns per op (DVE lost ~23us to save 13us).

    nc = bacc.Bacc("TRN2", target_bir_lowering=False, debug=False,
                   num_devices=NCORES)

    # hT and A arrive host-relaid so each SBUF partition's bytes are one
    # contiguous DRAM run (8KB descriptors instead of 1-2KB: 16x fewer
    # descriptors -> faster SWDGE descgen and lower DMA-queue occupancy)
    hT = nc.dram_tensor("hT", [128, 2 * N], bf16, kind="ExternalInput").ap()
    Ab = nc.dram_tensor("Ab", [NCH * 128, NJ * CH], bf16,
                        kind="ExternalInput").ap()
    # all weights/biases packed in one buffer: each dma_start costs ~620ns
    # of serial sync-sequencer issue regardless of size, so one fat DMA
    # beats five thin ones.  bf16 cols: wq[s0|s1] 0:256, wk 256:384,
    # wv 384:512, bq4(f32) 512:514, bvb(f32) 514:642.
    WPK = 4 * D * 2 + HPC * D * 2 * 2 + 2 + HPC * D * 2
    wpack = nc.dram_tensor("wpack", [128, WPK], bf16,
                           kind="ExternalInput").ap()
    o = nc.dram_tensor("o", [128, N], f32, kind="ExternalOutput").ap()

    SC = 1.0 / math.sqrt(D)

    def bcast_free(ap_col, n):
        return bass.AP(tensor=ap_col.tensor, offset=ap_col.offset,
                       ap=[ap_col.ap[0], [0, n]])

    # block index for (head hh, j-tile t): strip r = 2*hh + t%2
    def blk(hh, t):
        return 4 * (t // 2) + 2 * hh + (t % 2)

    with (
        tile.TileContext(nc) as tc,
        tc.tile_pool(name="const", bufs=1) as cpool,
        tc.tile_pool(name="ps", bufs=2, space="PSUM") as pspool,
        tc.tile_pool(name="pod", bufs=2, space="PSUM") as podpool,
        tc.tile_pool(name="apool", bufs=2) as apool,
        tc.tile_pool(name="epool", bufs=2) as epool,
        tc.tile_pool(name="eapool", bufs=2) as eapool,
        tc.tile_pool(name="opool", bufs=2) as opool,
    ):
        # ---- constants / inputs into SBUF
        scratch = cpool.tile([128, CH], bf16, tag="scratch")
        nc.vector.memset(scratch, 0.0)
        ones_sb = cpool.tile([128, 1], bf16, tag="ones")
        nc.gpsimd.memset(ones_sb, 1.0)
        # dummy activation to pull the ~1.3us exp ACT_TABLE_LOAD into the
        # preamble instead of serializing before the first real exp
        warm_sb = cpool.tile([128, 1], bf16, tag="warm")
        nc.scalar.activation(warm_sb, ones_sb, AF.Exp)

        # startup DMAs in need-order (per-queue rings are FIFO in posting
        # order, so later DMAs queue behind earlier ones): Q/K-proj columns
        # of hT, the packed weights, the rest of hT half 0, the hT half-1
        # head for K-proj c1, then the chunk-0 A tile in two halves so the
        # early pairs' EA can start while the second half streams.
        # hT DRAM layout is column-blocked: [p, q(4), s(2), 512] so each
        # 512-col piece is ONE 2KB run per partition (128 fat descriptors)
        hT4 = hT.rearrange("p (q s n) -> p q s n", q=4, s=2)
        hT_half = [cpool.tile([128, 2, N // 2], bf16, tag=f"hT{c}",
                              name=f"hT{c}")
                   for c in range(2)]
        wpk_sb = cpool.tile([128, WPK], bf16, tag="wpk")
        nc.sync.dma_start(hT_half[0][:, :, 0:CH], hT4[:, 0])
        nc.sync.dma_start(wpk_sb, wpack)
        nc.sync.dma_start(hT_half[0][:, :, CH:2 * CH], hT4[:, 1])
        nc.sync.dma_start(hT_half[1][:, :, 0:CH], hT4[:, 2])

        _woff = {"q": (0, 4 * D), "k": (2 * 4 * D, HPC * D),
                 "v": (2 * 4 * D + 2 * HPC * D, HPC * D)}

        def wcols(name, s):
            off, m = _woff[name]
            return wpk_sb[:, off + s * m:off + (s + 1) * m]

        bq4_sb = wpk_sb[:, 512:514].bitcast(f32)
        bvb_sb = wpk_sb[:, 514:642].bitcast(f32)

        def hT_sb_cols(s, lo, hi):       # [lo, hi) within one half
            c = lo // (N // 2)
            assert (hi - 1) // (N // 2) == c
            return hT_half[c][:, s, lo - c * N // 2:hi - c * N // 2]

        qt4 = cpool.tile([128, N], bf16, tag="qt4")      # strips [h0|h0|h1|h1]
        kt4 = cpool.tile([128, NJ // 2 * 128], bf16, tag="kt4")
        Vt = cpool.tile([128, NJ * HPC * D], bf16, tag="vt")   # col = t*64+d

        A3 = Ab.rearrange("(c p) (t i) -> c p t i", p=128, i=CH)
        a_tiles = [None] * NCH

        def emit_a_dma(ch):
            a_t = apool.tile([128, NJ, CH], bf16, tag="a")
            nc.sync.dma_start(a_t, A3[ch])
            a_tiles[ch] = a_t

        # chunk-0 A in two halves bracketing the hT half-1 tail so the
        # K-proj c1 / late V-proj columns aren't stuck behind 2.1MB of A
        a0_t = apool.tile([128, NJ, CH], bf16, tag="a")
        nc.sync.dma_start(hT_half[1][:, :, CH:2 * CH], hT4[:, 3])
        nc.sync.dma_start(a0_t[:, 0:NJ // 2, :], A3[0][:, 0:NJ // 2])
        nc.sync.dma_start(a0_t[:, NJ // 2:, :], A3[0][:, NJ // 2:])
        a_tiles[0] = a0_t

        # ---- PE warm-up: a few dummy matmuls while input DMAs are in
        #      flight (kept small: a dense burst across all 8 cores trips
        #      the chip-wide power throttle and downclocks everything 1.2x)
        for _ in range(3):
            jt = podpool.tile([128, CH], f32, tag="od", name="junk")
            nc.tensor.matmul(jt, lhsT=scratch[:, 0:128], rhs=scratch,
                             start=True, stop=True)

        # ---- V projection helpers (all 16 tiles run inline in chunk 0)
        def emit_vproj_mms(vps, base, t0, cnt):
            for t in range(t0, t0 + cnt):
                for s in range(2):
                    nc.tensor.matmul(
                        vps[:, (t - base) * HPC * D:(t - base + 1) * HPC * D],
                        lhsT=hT_sb_cols(s, t * 128, (t + 1) * 128),
                        rhs=wcols("v", s),
                        start=(s == 0), stop=(s == 1),
                    )

        def emit_vproj_add(vps, t0, cnt=8):
            vsl = vps[:, (t0 % 8) * HPC * D:(t0 % 8 + cnt) * HPC * D]
            base = Vt[:, t0 * HPC * D:(t0 + cnt) * HPC * D]
            out_ap = bass.AP(tensor=base.tensor, offset=base.offset,
                             ap=[base.ap[0], [HPC * D, cnt], [1, HPC * D]])
            in_ap = bass.AP(tensor=vsl.tensor, offset=vsl.offset,
                            ap=[vsl.ap[0], [HPC * D, cnt], [1, HPC * D]])
            b_ap = bass.AP(tensor=bvb_sb.tensor, offset=bvb_sb.offset,
                           ap=[bvb_sb.ap[0], [0, cnt], [1, HPC * D]])
            nc.vector.tensor_add(out_ap, in_ap, b_ap)

        # ---- K projection into packed strip layout.
        #      strip r holds K^T of head r//2 for tiles t = 2*q2 + r%2.
        #      c=0 (j-tiles 0-7 -> S groups 0-4) runs on the first hT half
        #      before the loop; c=1 is emitted inside chunk 0, group 0.
        def emit_kproj(c):
            kps = pspool.tile([128, CH], f32, tag="ps", name=f"kps{c}")
            for r in range(4):
                for s in range(2):
                    base = hT_half[c][:, s, (r % 2) * 128:(r % 2) * 128 + 128]
                    rhs = bass.AP(tensor=base.tensor, offset=base.offset,
                                  ap=[base.ap[0], [256, 4], [1, 128]])
                    nc.tensor.matmul(
                        kps[32 * r:32 * r + 32, :],
                        lhsT=wcols("k", s)[:, (r // 2) * D:(r // 2 + 1) * D],
                        rhs=rhs,
                        start=(s == 0), stop=(s == 1),
                        tile_position=(0, 32 * r),
                    )
            return kps

        # bk dropped (cancels in softmax); cast on DVE so it runs in
        # parallel with the Q-proj eviction on ACT (startup critical path)
        kps0 = emit_kproj(0)
        nc.vector.tensor_copy(kt4[:, 0:CH], kps0[:, 0:CH])

        # ---- Q projection (+bias, scaled 1/sqrt(D)), strip-replicated
        #      directly: wq columns are [h0|h0|h1|h1] (host-packed).
        def emit_qproj(quarter, on_act=False):
            sl = slice(quarter * CH, (quarter + 1) * CH)
            qps = pspool.tile([128, CH], f32, tag="ps", name="qps")
            for s in range(2):
                nc.tensor.matmul(qps, lhsT=wcols("q", s),
                                 rhs=hT_sb_cols(s, quarter * CH,
                                                (quarter + 1) * CH),
                                 start=(s == 0), stop=(s == 1))
            if on_act:
                nc.scalar.activation(qt4[:, sl], qps, AF.Identity,
                                     bias=bq4_sb, scale=SC)
            else:
                nc.vector.scalar_tensor_tensor(
                    qt4[:, sl], qps, SC, bcast_free(bq4_sb, CH),
                    op0=mybir.AluOpType.mult, op1=mybir.AluOpType.add,
                )

        emit_qproj(0, on_act=True)

        # ---- main loop helpers
        def emit_amult(e_t, ea_t, a_t, p, hh):
            engine = nc.vector
            if hh is None:   # both heads of pair p in one op (4D AP)
                b0 = 4 * p * CH
                eb = e_t[:, b0:b0 + CH]
                e_ap = bass.AP(tensor=eb.tensor, offset=eb.offset,
                               ap=[eb.ap[0], [2 * CH, 2], [CH, 2], [1, CH]])
                eab = ea_t[:, b0:b0 + CH]
                ea_ap = bass.AP(tensor=eab.tensor, offset=eab.offset,
                                ap=[eab.ap[0], [2 * CH, 2], [CH, 2], [1, CH]])
                ab = a_t[:, 2 * p, :]
                a_ap = bass.AP(tensor=ab.tensor, offset=ab.offset,
                               ap=[ab.ap[0], [0, 2], [CH, 2], [1, CH]])
                engine.tensor_mul(ea_ap, e_ap, a_ap)
                return
            # blocks 4p+2hh, 4p+2hh+1 = head hh, j-tiles 2p, 2p+1: contiguous
            b0 = (4 * p + 2 * hh) * CH
            eb = e_t[:, b0:b0 + CH]
            e_ap = bass.AP(tensor=eb.tensor, offset=eb.offset,
                           ap=[eb.ap[0], [CH, 2], [1, CH]])
            eab = ea_t[:, b0:b0 + CH]
            ea_ap = bass.AP(tensor=eab.tensor, offset=eab.offset,
                            ap=[eab.ap[0], [CH, 2], [1, CH]])
            engine.tensor_mul(ea_ap, e_ap, a_t[:, 2 * p:2 * p + 2, :])

        def emit_ph2_quad(od, e_t, ea_t, p, hh, den_first=False):
            # NOTE: splitting these 128-contraction matmuls into 4 x 32-row
            # bands at tile positions (32r, col) -- to overlap like the S
            # strips do -- fails at runtime (INTERNAL error), so the
            # full-contraction form stays.
            def den(tp):
                t = 2 * p + tp
                bsl = slice(blk(hh, t) * CH, (blk(hh, t) + 1) * CH)
                nc.tensor.matmul(
                    od[64 + 32 * hh:65 + 32 * hh, :],
                    lhsT=ones_sb, rhs=e_t[:, bsl],
                    start=(t == 0), stop=(t == NJ - 1),
                    tile_position=(0, 64 + 32 * hh),
                )

            def vmm(tp):
                t = 2 * p + tp
                bsl = slice(blk(hh, t) * CH, (blk(hh, t) + 1) * CH)
                nc.tensor.matmul(
                    od[32 * hh:32 * hh + 32, :],
                    lhsT=Vt[:, t * 64 + 32 * hh:t * 64 + 32 * hh + 32],
                    rhs=ea_t[:, bsl],
                    start=(t == 0), stop=(t == NJ - 1),
                    tile_position=(0, 32 * hh),
                )

            if den_first:   # dens only need E, not the DVE product
                den(0), den(1), vmm(0), vmm(1)
            else:
                vmm(0), den(0), vmm(1), den(1)

        def emit_out(od, ch):
            # PSUM->SBUF eviction on ACT (measured better than DVE here);
            # DMA cannot read PSUM directly.
            o_sb = opool.tile([128, CH], f32, tag="o")
            nc.scalar.activation(o_sb, od, AF.Copy)
            nc.sync.dma_start(o[:, ch * CH:(ch + 1) * CH], o_sb)

        carry = None   # (od, e_t, ea_t, ch, quads) spill of previous chunk
        for ch in range(NCH):
            if ch + 1 < NCH:
                emit_a_dma(ch + 1)
            a_t = a_tiles[ch]
            e_t = epool.tile([128, NBLK * CH], bf16, tag="e")
            ea_t = eapool.tile([128, NBLK * CH], bf16, tag="ea")
            od = None

            # slot schedule: one slot per pair p.  Each slot computes the
            # pair's 4 S blocks: blocks 4p,4p+1 into a 2-bank "ps" tile and
            # 4p+2(,4p+3) into another, all consumed by ACT exp; for
            # Schraudolph pairs block 4p+3 goes to a separate 1-bank "psv"
            # tile consumed by DVE, so the two PSUM rotations never couple
            # the ACT exp stream to the saturated DVE queue.  The pair's
            # merged EA tensor_tensor follows its Schraudolph in the same
            # DVE queue; phase2 quads trail by l0 slots and spill into the
            # next chunk's slots 0-2.
            last = ch == NCH - 1
            sch_pairs = SCHRAUD[ch] if ch < len(SCHRAUD) else ()
            amult_at = {}
            epilogue_amults = []
            ph2_at = {}
            spill = []

            # od accumulation (start at t=0) requires quads of one head to
            # hit the PE queue in j-tile order: clamp to the head's running
            # max slot (append order within a slot is p-ascending).
            last_gq = [0, 0]

            def sched_quad(gq, p, hh):
                gq = max(gq, last_gq[hh])
                last_gq[hh] = gq
                if gq < NPAIR:
                    ph2_at.setdefault(gq, []).append((p, hh))
                else:
                    spill.append((p, hh))

            for p in range(NPAIR):
                schr = p in sch_pairs
                base_slot = p + 1 if schr else p
                if ch == 0:
                    # chunk-0 EA waits the A0 halves landing (~slots 2/4)
                    base_slot = max(base_slot, 2 if p < 4 else p)
                if last and p >= 6:      # tail: per-head DVE TTs, min lag
                    for hh in range(HPC):
                        amult_at.setdefault(p, []).append((p, hh))
                        sched_quad(min(p + 1, NPAIR - 1), p, hh)
                else:
                    if base_slot < NPAIR:
                        amult_at.setdefault(base_slot, []).append((p, None))
                    else:
                        epilogue_amults.append((p, None))
                    lag = 2 if last else 3
                    for hh in range(HPC):
                        sched_quad(base_slot + lag + hh, p, hh)

            def emit_s(dest, b):
                q2, r = b // 4, b % 4
                nc.tensor.matmul(
                    dest,
                    lhsT=kt4[32 * r:32 * r + 32, q2 * 128:(q2 + 1) * 128],
                    rhs=qt4[32 * r:32 * r + 32, ch * CH:(ch + 1) * CH],
                    start=True, stop=True,
                    tile_position=(32 * r, 0),
                )

            def emit_schr_pair(pp):
                # 1-buf psv tile, PE-fill deferred one slot: schraud(pp-2)
                # has ~3 slots to run before this fill waits on it
                tv = pspool.tile([128, 2 * CH], f32, tag="psv", bufs=1,
                                 name="sv")
                emit_s(tv[:, 0:CH], 4 * pp + 2)
                emit_s(tv[:, CH:], 4 * pp + 3)
                nc.vector.tensor_scalar(
                    e_t[:, (4 * pp + 2) * CH:(4 * pp + 4) * CH].bitcast(i16),
                    tv, SCH_A, SCH_B,
                    op0=mybir.AluOpType.mult, op1=mybir.AluOpType.add)

            pending_schr = None
            for p in range(NPAIR):
                b0 = 4 * p
                schr = p in sch_pairs
                ta = pspool.tile([128, 2 * CH], f32, tag="ps", name="sA")
                emit_s(ta[:, 0:CH], b0)
                emit_s(ta[:, CH:], b0 + 1)
                if not schr:
                    tb = pspool.tile([128, 2 * CH], f32, tag="ps", name="sB")
                    emit_s(tb[:, 0:CH], b0 + 2)
                    emit_s(tb[:, CH:], b0 + 3)
                if pending_schr is not None:
                    emit_schr_pair(pending_schr)
                    pending_schr = None
                if schr:
                    pending_schr = p

                nc.scalar.activation(e_t[:, b0 * CH:(b0 + 2) * CH], ta,
                                     AF.Exp)
                if not schr:
                    nc.scalar.activation(
                        e_t[:, (b0 + 2) * CH:(b0 + 4) * CH], tb, AF.Exp)

                for pp, hh in amult_at.get(p, ()):
                    emit_amult(e_t, ea_t, a_t, pp, hh)

                # drain previous chunk's spilled phase2 quads: 2 per slot,
                # finishing at slot 2
                if carry is not None and p <= 2:
                    cod, ce, cea, cch, cquads = carry
                    take = cquads[:2] if p < 2 else cquads
                    for pp, hh in take:
                        emit_ph2_quad(cod, ce, cea, pp, hh)
                    cquads = cquads[len(take):]
                    if not cquads:
                        emit_out(cod, cch)
                        carry = None
                    else:
                        carry = (cod, ce, cea, cch, cquads)

                if ch == 0:
                    if p == 1:
                        # K projection second half at slot 1 (hT half 1
                        # lands ~slot 0.5; emitting at slot 0 head-of-line
                        # blocks the PE queue on the DMA); cast on ACT so
                        # the "ps" rotation's consumers stay single-engine
                        kps1 = emit_kproj(1)
                        nc.scalar.activation(kt4[:, CH:2 * CH], kps1,
                                             AF.Copy)
                    if p < 4:
                        if p == 0:
                            vps0 = podpool.tile([128, CH], f32, tag="od",
                                                name="vps0")
                        emit_vproj_mms(vps0, 0, 2 * p, 2)
                        if p == 1:
                            emit_vproj_add(vps0, 0, 4)
                        elif p == 3:
                            emit_vproj_add(vps0, 4, 4)
                    else:
                        if p == 4:
                            vps1 = podpool.tile([128, CH], f32, tag="od",
                                                name="vps1")
                        emit_vproj_mms(vps1, 8, 2 * p, 2)
                        if p == 5:
                            emit_vproj_add(vps1, 8, 4)
                        elif p == 7:
                            emit_vproj_add(vps1, 12, 4)
                    if p == 5:
                        emit_qproj(1, on_act=True)
                elif ch in (1, 2) and p == 5:
                    emit_qproj(ch + 1, on_act=True)

                for pp, hh in ph2_at.get(p, ()):
                    if od is None:
                        od = podpool.tile([128, CH], f32, tag="od")
                    emit_ph2_quad(od, e_t, ea_t, pp, hh)

            # chunk epilogue: pair-7's deferred Schraudolph + EA
            if pending_schr is not None:
                emit_schr_pair(pending_schr)
                pending_schr = None
            for pp, hh in epilogue_amults:
                emit_amult(e_t, ea_t, a_t, pp, hh)

            carry = (od, e_t, ea_t, ch, spill)

        cod, ce, cea, cch, cquads = carry
        for p, hh in cquads:
            emit_ph2_quad(cod, ce, cea, p, hh)
        emit_out(cod, cch)

    nc.finalize()
    return nc


def kernel(h, A, Wq, bq, Wk, bk, Wv, bv):
    global LAST_RESULTS
    from concourse.bass_utils import run_bass_kernel_spmd

    h = np.asarray(h, np.float32)
    A = np.asarray(A, np.float32)
    Wq = np.asarray(Wq, np.float32)
    Wk = np.asarray(Wk, np.float32)
    Wv = np.asarray(Wv, np.float32)
    bq = np.asarray(bq, np.float32)
    bv = np.asarray(bv, np.float32)

    # hT: [b, 128(p), q(4), s(2), 512] column-blocked so each 512-col DMA
    # piece is one contiguous 2KB run per partition
    hT = (h.transpose(0, 2, 1)                     # [b, IN_DIM, N]
          .reshape(B, 2, 128, N).transpose(0, 2, 1, 3)
          .reshape(B, 128, 2, 4, 512).transpose(0, 1, 3, 2, 4)
          .reshape(B, 128, 2 * N))
    hT = np.ascontiguousarray(hT).astype(ml_dtypes.bfloat16)
    # A: [b, ch*128(p), t*CH(i)]: a_t[p, t, i] = A[b, t*128+p, ch*CH+i]
    Ab = (A.reshape(B, NJ, 128, NCH, CH).transpose(0, 3, 2, 1, 4)
          .reshape(B, NCH * 128, NJ * CH))
    Ab = np.ascontiguousarray(Ab).astype(ml_dtypes.bfloat16)
    sc = np.float32(1.0 / math.sqrt(D))

    in_maps = []
    for c in range(NCORES):
        b = c // CORES_PER_B
        h0 = HPC * (c % CORES_PER_B)
        sl = slice(h0 * D, (h0 + HPC) * D)
        wq_h = [Wq[:, (h0 + k) * D:(h0 + k + 1) * D] for k in range(HPC)]
        wq_rep = np.concatenate([wq_h[0], wq_h[0], wq_h[1], wq_h[1]], axis=1)
        bq_h = [bq[(h0 + k) * D:(h0 + k + 1) * D] for k in range(HPC)]
        bq4 = np.concatenate([bq_h[0], bq_h[0], bq_h[1], bq_h[1]]) * sc
        # one packed [128, 642] bf16 buffer: wq s0|s1, wk s0|s1, wv s0|s1
        # (each [128, m] slab), then bq4/bvb as raw f32 bytes
        slabs = []
        for w in (wq_rep, Wk[:, sl], Wv[:, sl]):
            wb = w.astype(ml_dtypes.bfloat16).view(np.uint16)
            slabs += [wb[0:128, :], wb[128:256, :]]
        slabs.append(np.ascontiguousarray(
            bq4.reshape(128, 1).astype(np.float32)).view(np.uint16))
        slabs.append(np.ascontiguousarray(
            np.tile(bv[sl][None, :], (128, 1)).astype(np.float32))
            .view(np.uint16))
        wpk = np.ascontiguousarray(
            np.concatenate(slabs, axis=1)).view(ml_dtypes.bfloat16)
        in_maps.append({
            "hT": hT[b],
            "Ab": Ab[b],
            "wpack": wpk,
        })

    nc = _build_bass()
    res = run_bass_kernel_spmd(
        nc, in_maps, core_ids=list(range(NCORES)),
        trace=os.environ.get("BASS_TRACE", "0") == "1",
    )
    LAST_RESULTS = res

    out = np.empty((B, HEADS, N, D), np.float32)
    for c in range(NCORES):
        b = c // CORES_PER_B
        h0 = HPC * (c % CORES_PER_B)
        oo = res.results[c]["o"]                  # [128, N] f32
        for hh in range(HPC):
            num = oo[hh * D:(hh + 1) * D, :]      # [32, N] unnormalized out^T
            den = oo[64 + 32 * hh, :]             # [N]
            out[b, h0 + hh] = (num / den[None, :]).T
    return out



# revision 46
# speedup vs baseline: 1.0398x; 1.0032x over previous
"""Adjacency-aware multi-head attention on 8 trn2 NeuronCores.

Math (per b, head k):
  Q = h[b] @ Wq[:, k] + bq[k]           [N, D]
  S[i, j] = (Q_i . K_j) / sqrt(D)
  P[j, i] = exp(S[i, j]) / sum_j exp(S[i, j])      (softmax over keys j)
  out[i, d] = sum_j P[j, i] * A[b, j, i] * V[j, d]

The K bias cancels: it adds g[i] = Q_i . bk to every score of query i,
and softmax over j is invariant to per-i shifts -> bk is dropped.

Sharding: 16 (b, head) pairs over 8 cores, 2 heads of the SAME b per core so
the A[b] stream is shared by both heads.

Device dataflow ([j, i] layout so A needs no transpose).  exp of all scores
(8.4M elem/core) is the dominant elementwise cost; it is SPLIT between the
ACT engine (exact exp) and the DVE (Schraudolph int16-bitcast affine approx,
runs at DVE 2x mode: ~0.58ns/col) so both engines land at ~58us busy:
  - Strip mapping r = 2*head + (t%2): j-tile t of head hh computes on PE row
    strip r.  Q^T is written strip-replicated directly by the Q projection
    (host passes Wq with columns [h0|h0|h1|h1]), no SBUF->SBUF copies.
    The K bias is dropped (cancels in softmax).
  - Slot structure: one slot per j-tile pair p (4 S blocks).  ACT pairs'
    blocks go through a 2-buf 2-bank "ps" PSUM tag consumed ONLY by ACT;
    Schraudolph pairs' blocks 4p+2,4p+3 go through a 1-buf "psv" tag whose
    PE fill is deferred one slot, so the saturated DVE queue can lag ~3
    slots without ever stalling the ACT exp rotation (coupling the two
    rotations was measured to cost ~10us in boundary stalls).
  - The pair's merged EA = E * A tensor_tensor (one 4D-AP op, 2x bf16)
    follows its Schraudolph in the same DVE queue -> no cross-engine wait.
  - Startup: each dma_start costs ~620ns of SERIAL sync-sequencer issue and
    per-queue rings are FIFO in posting order, so inputs are packed into 7
    fat DMAs in need-order (hT pieces are single 2KB runs per partition via
    a column-blocked host layout; all weights/biases in one "wpack" buffer;
    chunk-0 A split in two halves).  A dummy exp pulls the ~1.3us
    ACT_TABLE_LOAD into the preamble.  A few dummy matmuls warm the PE
    clock gate (kept small: a dense burst across 8 cores trips the
    chip-wide power throttle, which also makes run-to-run timing vary
    by ~10-20%).
  - phase 2 quads trail ~2 slots behind, 4 column-tiled streams into one
    PSUM tile: out_h0 (rows 0-31), out_h1 (32-63), denom_h0 (row 64),
    denom_h1 (row 96); late quads spill into the next chunk's slots 0-2.
    PSUM->SBUF output eviction runs on ACT (Copy) to keep DVE clear.
Device returns [128, N]: rows 0-31 outT_h0, 32-63 outT_h1, rows 64/96 the
softmax denominators.  Host does out = (outT / denom)^T plus the gather.

BASS_SCHR picks the per-chunk Schraudolph pairs (default 4;4;4;3 of 8 --
~23% of elements approximated, end-to-end rel err ~4.3e-3 vs 3.4e-3 exact).
"""

import math
import os

import numpy as np
import ml_dtypes

B, N, IN_DIM = 2, 2048, 256
HEADS, D = 8, 32
NCORES = 8
HPC = 2              # heads per core
NJ = N // 128        # 16 j-tiles
NCH = 4              # i-chunks
CH = N // NCH        # 512
CORES_PER_B = NCORES // B
GRP = 3              # S psum banks per exp op
NBLK = NJ * HPC      # 32 S blocks per chunk
NGRP = (NBLK + GRP - 1) // GRP   # 11 exp groups per chunk
NPAIR = NJ // 2      # 8 j-tile pairs

LAST_RESULTS = None  # BassKernelResults of the most recent kernel() call


def _build_bass():
    import concourse.bass as bass
    import concourse.mybir as mybir
    import concourse.tile as tile
    from concourse import bacc

    f32 = mybir.dt.float32
    bf16 = mybir.dt.bfloat16
    i16 = mybir.dt.int16
    AF = mybir.ActivationFunctionType

    # Schraudolph exp: bf16 bit pattern of exp(x) ~ int16(x*128*log2(e) +
    # 127*128 - c).  Piecewise-linear 2^frac approx, max rel err ~3.4%;
    # softmax normalization cancels most of it (numpy sim of the full
    # pipeline: ~1.0e-2 end-to-end at 25% of groups approximated, vs the
    # 2e-2 gate).  Offloads ACT -> DVE.
    SCH_A = 128 * 1.4426950408889634
    SCH_B = 127.0 * 128 - 7.0
    # per-chunk exp groups computed on DVE via Schraudolph instead of ACT
    # per-chunk ODD pairs whose blocks 4p+2,4p+3 are Schraudolphed on DVE
    # via a dedicated 1-buf "psv" PSUM tag (deferred fill) so the ACT exp
    # rotation never waits on the saturated DVE queue; last chunk keeps
    # pair 7 on ACT so the drain isn't DVE-gated
    _schr = os.environ.get("BASS_SCHR", "1,3,5,7;1,3,5,7;1,3,5,7;1,3,5")
    SCHRAUD = [tuple(int(x) for x in part.split(",") if x != "")
               for part in _schr.split(";")]
    # NOTE: offloading EA tensor_tensors to GPSIMD was measured
    # NET-NEGATIVE on hardware: GPSIMD streams through the DVE's second
    # SBUF port, and concurrent GPS TTs degrade DVE tensor_tensor from
    # ~1133ns to ~1430-2731ns per op (DVE lost ~23us to save 13us).

    nc = bacc.Bacc("TRN2", target_bir_lowering=False, debug=False,
                   num_devices=NCORES)

    # hT and A arrive host-relaid so each SBUF partition's bytes are one
    # contiguous DRAM run (8KB descriptors instead of 1-2KB: 16x fewer
    # descriptors -> faster SWDGE descgen and lower DMA-queue occupancy)
    hT = nc.dram_tensor("hT", [128, 2 * N], bf16, kind="ExternalInput").ap()
    Ab = nc.dram_tensor("Ab", [NCH * 128, NJ * CH], bf16,
                        kind="ExternalInput").ap()
    # all weights/biases packed in one buffer: each dma_start costs ~620ns
    # of serial sync-sequencer issue regardless of size, so one fat DMA
    # beats five thin ones.  bf16 cols: wq[s0|s1] 0:256, wk 256:384,
    # wv 384:512, bq4(f32) 512:514, bvb(f32) 514:642.
    WPK = 4 * D * 2 + HPC * D * 2 * 2 + 2 + HPC * D * 2
    wpack = nc.dram_tensor("wpack", [128, WPK], bf16,
                           kind="ExternalInput").ap()
    o = nc.dram_tensor("o", [128, N], f32, kind="ExternalOutput").ap()

    SC = 1.0 / math.sqrt(D)

    def bcast_free(ap_col, n):
        return bass.AP(tensor=ap_col.tensor, offset=ap_col.offset,
                       ap=[ap_col.ap[0], [0, n]])

    # block index for (head hh, j-tile t): strip r = 2*hh + t%2
    def blk(hh, t):
        return 4 * (t // 2) + 2 * hh + (t % 2)

    with (
        tile.TileContext(nc) as tc,
        tc.tile_pool(name="const", bufs=1) as cpool,
        tc.tile_pool(name="ps", bufs=2, space="PSUM") as pspool,
        tc.tile_pool(name="pod", bufs=2, space="PSUM") as podpool,
        tc.tile_pool(name="apool", bufs=2) as apool,
        tc.tile_pool(name="epool", bufs=2) as epool,
        tc.tile_pool(name="eapool", bufs=2) as eapool,
        tc.tile_pool(name="opool", bufs=2) as opool,
    ):
        # ---- constants / inputs into SBUF
        scratch = cpool.tile([128, CH], bf16, tag="scratch")
        nc.vector.memset(scratch, 0.0)
        ones_sb = cpool.tile([128, 1], bf16, tag="ones")
        nc.gpsimd.memset(ones_sb, 1.0)
        # dummy activation to pull the ~1.3us exp ACT_TABLE_LOAD into the
        # preamble instead of serializing before the first real exp
        warm_sb = cpool.tile([128, 1], bf16, tag="warm")
        nc.scalar.activation(warm_sb, ones_sb, AF.Exp)

        # startup DMAs in need-order (per-queue rings are FIFO in posting
        # order, so later DMAs queue behind earlier ones): Q/K-proj columns
        # of hT, the packed weights, the rest of hT half 0, the hT half-1
        # head for K-proj c1, then the chunk-0 A tile in two halves so the
        # early pairs' EA can start while the second half streams.
        # hT DRAM layout is column-blocked: [p, q(4), s(2), 512] so each
        # 512-col piece is ONE 2KB run per partition (128 fat descriptors)
        hT4 = hT.rearrange("p (q s n) -> p q s n", q=4, s=2)
        hT_half = [cpool.tile([128, 2, N // 2], bf16, tag=f"hT{c}",
                              name=f"hT{c}")
                   for c in range(2)]
        wpk_sb = cpool.tile([128, WPK], bf16, tag="wpk")
        nc.sync.dma_start(hT_half[0][:, :, 0:CH], hT4[:, 0])
        nc.sync.dma_start(wpk_sb, wpack)
        nc.sync.dma_start(hT_half[0][:, :, CH:2 * CH], hT4[:, 1])
        nc.sync.dma_start(hT_half[1][:, :, 0:CH], hT4[:, 2])

        _woff = {"q": (0, 4 * D), "k": (2 * 4 * D, HPC * D),
                 "v": (2 * 4 * D + 2 * HPC * D, HPC * D)}

        def wcols(name, s):
            off, m = _woff[name]
            return wpk_sb[:, off + s * m:off + (s + 1) * m]

        bq4_sb = wpk_sb[:, 512:514].bitcast(f32)
        bvb_sb = wpk_sb[:, 514:642].bitcast(f32)

        def hT_sb_cols(s, lo, hi):       # [lo, hi) within one half
            c = lo // (N // 2)
            assert (hi - 1) // (N // 2) == c
            return hT_half[c][:, s, lo - c * N // 2:hi - c * N // 2]

        qt4 = cpool.tile([128, N], bf16, tag="qt4")      # strips [h0|h0|h1|h1]
        kt4 = cpool.tile([128, NJ // 2 * 128], bf16, tag="kt4")
        Vt = cpool.tile([128, NJ * HPC * D], bf16, tag="vt")   # col = t*64+d

        A3 = Ab.rearrange("(c p) (t i) -> c p t i", p=128, i=CH)
        a_tiles = [None] * NCH

        def emit_a_dma(ch):
            a_t = apool.tile([128, NJ, CH], bf16, tag="a")
            nc.sync.dma_start(a_t, A3[ch])
            a_tiles[ch] = a_t

        # chunk-0 A in two halves bracketing the hT half-1 tail so the
        # K-proj c1 / late V-proj columns aren't stuck behind 2.1MB of A
        a0_t = apool.tile([128, NJ, CH], bf16, tag="a")
        nc.sync.dma_start(hT_half[1][:, :, CH:2 * CH], hT4[:, 3])
        nc.sync.dma_start(a0_t[:, 0:NJ // 2, :], A3[0][:, 0:NJ // 2])
        nc.sync.dma_start(a0_t[:, NJ // 2:, :], A3[0][:, NJ // 2:])
        a_tiles[0] = a0_t

        # ---- PE warm-up: a few dummy matmuls while input DMAs are in
        #      flight (kept small: a dense burst across all 8 cores trips
        #      the chip-wide power throttle and downclocks everything 1.2x)
        for _ in range(3):
            jt = podpool.tile([128, CH], f32, tag="od", name="junk")
            nc.tensor.matmul(jt, lhsT=scratch[:, 0:128], rhs=scratch,
                             start=True, stop=True)

        # ---- V projection helpers (all 16 tiles run inline in chunk 0)
        def emit_vproj_mms(vps, base, t0, cnt):
            for t in range(t0, t0 + cnt):
                for s in range(2):
                    nc.tensor.matmul(
                        vps[:, (t - base) * HPC * D:(t - base + 1) * HPC * D],
                        lhsT=hT_sb_cols(s, t * 128, (t + 1) * 128),
                        rhs=wcols("v", s),
                        start=(s == 0), stop=(s == 1),
                    )

        def emit_vproj_add(vps, t0, cnt=8):
            vsl = vps[:, (t0 % 8) * HPC * D:(t0 % 8 + cnt) * HPC * D]
            base = Vt[:, t0 * HPC * D:(t0 + cnt) * HPC * D]
            out_ap = bass.AP(tensor=base.tensor, offset=base.offset,
                             ap=[base.ap[0], [HPC * D, cnt], [1, HPC * D]])
            in_ap = bass.AP(tensor=vsl.tensor, offset=vsl.offset,
                            ap=[vsl.ap[0], [HPC * D, cnt], [1, HPC * D]])
            b_ap = bass.AP(tensor=bvb_sb.tensor, offset=bvb_sb.offset,
                           ap=[bvb_sb.ap[0], [0, cnt], [1, HPC * D]])
            nc.vector.tensor_add(out_ap, in_ap, b_ap)

        # ---- K projection into packed strip layout.
        #      strip r holds K^T of head r//2 for tiles t = 2*q2 + r%2.
        #      c=0 (j-tiles 0-7 -> S groups 0-4) runs on the first hT half
        #      before the loop; c=1 is emitted inside chunk 0, group 0.
        def emit_kproj(c):
            kps = pspool.tile([128, CH], f32, tag="ps", name=f"kps{c}")
            for r in range(4):
                for s in range(2):
                    base = hT_half[c][:, s, (r % 2) * 128:(r % 2) * 128 + 128]
                    rhs = bass.AP(tensor=base.tensor, offset=base.offset,
                                  ap=[base.ap[0], [256, 4], [1, 128]])
                    nc.tensor.matmul(
                        kps[32 * r:32 * r + 32, :],
                        lhsT=wcols("k", s)[:, (r // 2) * D:(r // 2 + 1) * D],
                        rhs=rhs,
                        start=(s == 0), stop=(s == 1),
                        tile_position=(0, 32 * r),
                    )
            return kps

        # bk dropped (cancels in softmax); cast on DVE so it runs in
        # parallel with the Q-proj eviction on ACT (startup critical path)
        kps0 = emit_kproj(0)
        nc.vector.tensor_copy(kt4[:, 0:CH], kps0[:, 0:CH])

        # ---- Q projection (+bias, scaled 1/sqrt(D)), strip-replicated
        #      directly: wq columns are [h0|h0|h1|h1] (host-packed).
        def emit_qproj(quarter, on_act=False):
            sl = slice(quarter * CH, (quarter + 1) * CH)
            qps = pspool.tile([128, CH], f32, tag="ps", name="qps")
            for s in range(2):
                nc.tensor.matmul(qps, lhsT=wcols("q", s),
                                 rhs=hT_sb_cols(s, quarter * CH,
                                                (quarter + 1) * CH),
                                 start=(s == 0), stop=(s == 1))
            if on_act:
                nc.scalar.activation(qt4[:, sl], qps, AF.Identity,
                                     bias=bq4_sb, scale=SC)
            else:
                nc.vector.scalar_tensor_tensor(
                    qt4[:, sl], qps, SC, bcast_free(bq4_sb, CH),
                    op0=mybir.AluOpType.mult, op1=mybir.AluOpType.add,
                )

        emit_qproj(0, on_act=True)

        # ---- main loop helpers
        def emit_amult(e_t, ea_t, a_t, p, hh):
            engine = nc.vector
            if hh is None:   # both heads of pair p in one op (4D AP)
                b0 = 4 * p * CH
                eb = e_t[:, b0:b0 + CH]
                e_ap = bass.AP(tensor=eb.tensor, offset=eb.offset,
                               ap=[eb.ap[0], [2 * CH, 2], [CH, 2], [1, CH]])
                eab = ea_t[:, b0:b0 + CH]
                ea_ap = bass.AP(tensor=eab.tensor, offset=eab.offset,
                                ap=[eab.ap[0], [2 * CH, 2], [CH, 2], [1, CH]])
                ab = a_t[:, 2 * p, :]
                a_ap = bass.AP(tensor=ab.tensor, offset=ab.offset,
                               ap=[ab.ap[0], [0, 2], [CH, 2], [1, CH]])
                engine.tensor_mul(ea_ap, e_ap, a_ap)
                return
            # blocks 4p+2hh, 4p+2hh+1 = head hh, j-tiles 2p, 2p+1: contiguous
            b0 = (4 * p + 2 * hh) * CH
            eb = e_t[:, b0:b0 + CH]
            e_ap = bass.AP(tensor=eb.tensor, offset=eb.offset,
                           ap=[eb.ap[0], [CH, 2], [1, CH]])
            eab = ea_t[:, b0:b0 + CH]
            ea_ap = bass.AP(tensor=eab.tensor, offset=eab.offset,
                            ap=[eab.ap[0], [CH, 2], [1, CH]])
            engine.tensor_mul(ea_ap, e_ap, a_t[:, 2 * p:2 * p + 2, :])

        def emit_ph2_quad(od, e_t, ea_t, p, hh, den_first=False):
            # NOTE: splitting these 128-contraction matmuls into 4 x 32-row
            # bands at tile positions (32r, col) -- to overlap like the S
            # strips do -- fails at runtime (INTERNAL error), so the
            # full-contraction form stays.
            def den(tp):
                t = 2 * p + tp
                bsl = slice(blk(hh, t) * CH, (blk(hh, t) + 1) * CH)
                nc.tensor.matmul(
                    od[64 + 32 * hh:65 + 32 * hh, :],
                    lhsT=ones_sb, rhs=e_t[:, bsl],
                    start=(t == 0), stop=(t == NJ - 1),
                    tile_position=(0, 64 + 32 * hh),
                )

            def vmm(tp):
                t = 2 * p + tp
                bsl = slice(blk(hh, t) * CH, (blk(hh, t) + 1) * CH)
                nc.tensor.matmul(
                    od[32 * hh:32 * hh + 32, :],
                    lhsT=Vt[:, t * 64 + 32 * hh:t * 64 + 32 * hh + 32],
                    rhs=ea_t[:, bsl],
                    start=(t == 0), stop=(t == NJ - 1),
                    tile_position=(0, 32 * hh),
                )

            if den_first:   # dens only need E, not the DVE product
                den(0), den(1), vmm(0), vmm(1)
            else:
                vmm(0), den(0), vmm(1), den(1)

        def emit_out(od, ch):
            # PSUM->SBUF eviction on ACT (measured better than DVE here);
            # DMA cannot read PSUM directly.
            o_sb = opool.tile([128, CH], f32, tag="o")
            nc.scalar.activation(o_sb, od, AF.Copy)
            nc.sync.dma_start(o[:, ch * CH:(ch + 1) * CH], o_sb)

        carry = None   # (od, e_t, ea_t, ch, quads) spill of previous chunk
        for ch in range(NCH):
            if ch + 1 < NCH:
                emit_a_dma(ch + 1)
            a_t = a_tiles[ch]
            e_t = epool.tile([128, NBLK * CH], bf16, tag="e")
            ea_t = eapool.tile([128, NBLK * CH], bf16, tag="ea")
            od = None

            # slot schedule: one slot per pair p.  Each slot computes the
            # pair's 4 S blocks: blocks 4p,4p+1 into a 2-bank "ps" tile and
            # 4p+2(,4p+3) into another, all consumed by ACT exp; for
            # Schraudolph pairs block 4p+3 goes to a separate 1-bank "psv"
            # tile consumed by DVE, so the two PSUM rotations never couple
            # the ACT exp stream to the saturated DVE queue.  The pair's
            # merged EA tensor_tensor follows its Schraudolph in the same
            # DVE queue; phase2 quads trail by l0 slots and spill into the
            # next chunk's slots 0-2.
            last = ch == NCH - 1
            sch_pairs = SCHRAUD[ch] if ch < len(SCHRAUD) else ()
            amult_at = {}
            epilogue_amults = []
            ph2_at = {}
            spill = []

            # od accumulation (start at t=0) requires quads of one head to
            # hit the PE queue in j-tile order: clamp to the head's running
            # max slot (append order within a slot is p-ascending).
            last_gq = [0, 0]

            def sched_quad(gq, p, hh):
                gq = max(gq, last_gq[hh])
                last_gq[hh] = gq
                if gq < NPAIR:
                    ph2_at.setdefault(gq, []).append((p, hh))
                else:
                    spill.append((p, hh))

            for p in range(NPAIR):
                schr = p in sch_pairs
                base_slot = p + 1 if schr else p
                if ch == 0:
                    # chunk-0 EA waits the A0 halves landing (~slots 2/4)
                    base_slot = max(base_slot, 2 if p < 4 else p)
                if last and p >= 6:      # tail: per-head DVE TTs, min lag
                    for hh in range(HPC):
                        amult_at.setdefault(p, []).append((p, hh))
                        sched_quad(min(p + 1, NPAIR - 1), p, hh)
                else:
                    if base_slot < NPAIR:
                        amult_at.setdefault(base_slot, []).append((p, None))
                    else:
                        epilogue_amults.append((p, None))
                    lag = 2 if last else 3
                    for hh in range(HPC):
                        sched_quad(base_slot + lag + hh, p, hh)

            def emit_s(dest, b):
                q2, r = b // 4, b % 4
                nc.tensor.matmul(
                    dest,
                    lhsT=kt4[32 * r:32 * r + 32, q2 * 128:(q2 + 1) * 128],
                    rhs=qt4[32 * r:32 * r + 32, ch * CH:(ch + 1) * CH],
                    start=True, stop=True,
                    tile_position=(32 * r, 0),
                )

            def emit_schr_pair(pp):
                # 1-buf psv tile, PE-fill deferred one slot: schraud(pp-2)
                # has ~3 slots to run before this fill waits on it
                tv = pspool.tile([128, 2 * CH], f32, tag="psv", bufs=1,
                                 name="sv")
                emit_s(tv[:, 0:CH], 4 * pp + 2)
                emit_s(tv[:, CH:], 4 * pp + 3)
                nc.vector.tensor_scalar(
                    e_t[:, (4 * pp + 2) * CH:(4 * pp + 4) * CH].bitcast(i16),
                    tv, SCH_A, SCH_B,
                    op0=mybir.AluOpType.mult, op1=mybir.AluOpType.add)

            pending_schr = None
            for p in range(NPAIR):
                b0 = 4 * p
                schr = p in sch_pairs
                ta = pspool.tile([128, 2 * CH], f32, tag="ps", name="sA")
                emit_s(ta[:, 0:CH], b0)
                emit_s(ta[:, CH:], b0 + 1)
                if not schr:
                    tb = pspool.tile([128, 2 * CH], f32, tag="ps", name="sB")
                    emit_s(tb[:, 0:CH], b0 + 2)
                    emit_s(tb[:, CH:], b0 + 3)
                if pending_schr is not None:
                    emit_schr_pair(pending_schr)
                    pending_schr = None
                if schr:
                    pending_schr = p

                nc.scalar.activation(e_t[:, b0 * CH:(b0 + 2) * CH], ta,
                                     AF.Exp)
                if not schr:
                    nc.scalar.activation(
                        e_t[:, (b0 + 2) * CH:(b0 + 4) * CH], tb, AF.Exp)

                for pp, hh in amult_at.get(p, ()):
                    emit_amult(e_t, ea_t, a_t, pp, hh)

                # drain previous chunk's spilled phase2 quads: 2 per slot,
                # finishing at slot 3 (8 spill quads with lag 3)
                if carry is not None and p <= 3:
                    cod, ce, cea, cch, cquads = carry
                    take = cquads[:2] if p < 3 else cquads
                    for pp, hh in take:
                        emit_ph2_quad(cod, ce, cea, pp, hh)
                    cquads = cquads[len(take):]
                    if not cquads:
                        emit_out(cod, cch)
                        carry = None
                    else:
                        carry = (cod, ce, cea, cch, cquads)

                if ch == 0:
                    if p == 1:
                        # K projection second half at slot 1 (hT half 1
                        # lands ~slot 0.5; emitting at slot 0 head-of-line
                        # blocks the PE queue on the DMA); cast on ACT so
                        # the "ps" rotation's consumers stay single-engine
                        kps1 = emit_kproj(1)
                        nc.scalar.activation(kt4[:, CH:2 * CH], kps1,
                                             AF.Copy)
                    if p < 4:
                        if p == 0:
                            vps0 = podpool.tile([128, CH], f32, tag="od",
                                                name="vps0")
                        emit_vproj_mms(vps0, 0, 2 * p, 2)
                        if p == 1:
                            emit_vproj_add(vps0, 0, 4)
                        elif p == 3:
                            emit_vproj_add(vps0, 4, 4)
                    else:
                        if p == 4:
                            vps1 = podpool.tile([128, CH], f32, tag="od",
                                                name="vps1")
                        emit_vproj_mms(vps1, 8, 2 * p, 2)
                        if p == 5:
                            emit_vproj_add(vps1, 8, 4)
                        elif p == 7:
                            emit_vproj_add(vps1, 12, 4)
                    if p == 5:
                        emit_qproj(1, on_act=True)
                elif ch in (1, 2) and p == 5:
                    emit_qproj(ch + 1, on_act=True)

                for pp, hh in ph2_at.get(p, ()):
                    if od is None:
                        od = podpool.tile([128, CH], f32, tag="od")
                    emit_ph2_quad(od, e_t, ea_t, pp, hh)

            # chunk epilogue: pair-7's deferred Schraudolph + EA
            if pending_schr is not None:
                emit_schr_pair(pending_schr)
                pending_schr = None
            for pp, hh in epilogue_amults:
                emit_amult(e_t, ea_t, a_t, pp, hh)

            carry = (od, e_t, ea_t, ch, spill)

        cod, ce, cea, cch, cquads = carry
        for p, hh in cquads:
            emit_ph2_quad(cod, ce, cea, p, hh)
        emit_out(cod, cch)

    nc.finalize()
    return nc


def kernel(h, A, Wq, bq, Wk, bk, Wv, bv):
    global LAST_RESULTS
    from concourse.bass_utils import run_bass_kernel_spmd

    h = np.asarray(h, np.float32)
    A = np.asarray(A, np.float32)
    Wq = np.asarray(Wq, np.float32)
    Wk = np.asarray(Wk, np.float32)
    Wv = np.asarray(Wv, np.float32)
    bq = np.asarray(bq, np.float32)
    bv = np.asarray(bv, np.float32)

    # hT: [b, 128(p), q(4), s(2), 512] column-blocked so each 512-col DMA
    # piece is one contiguous 2KB run per partition
    hT = (h.transpose(0, 2, 1)                     # [b, IN_DIM, N]
          .reshape(B, 2, 128, N).transpose(0, 2, 1, 3)
          .reshape(B, 128, 2, 4, 512).transpose(0, 1, 3, 2, 4)
          .reshape(B, 128, 2 * N))
    hT = np.ascontiguousarray(hT).astype(ml_dtypes.bfloat16)
    # A: [b, ch*128(p), t*CH(i)]: a_t[p, t, i] = A[b, t*128+p, ch*CH+i]
    Ab = (A.reshape(B, NJ, 128, NCH, CH).transpose(0, 3, 2, 1, 4)
          .reshape(B, NCH * 128, NJ * CH))
    Ab = np.ascontiguousarray(Ab).astype(ml_dtypes.bfloat16)
    sc = np.float32(1.0 / math.sqrt(D))

    in_maps = []
    for c in range(NCORES):
        b = c // CORES_PER_B
        h0 = HPC * (c % CORES_PER_B)
        sl = slice(h0 * D, (h0 + HPC) * D)
        wq_h = [Wq[:, (h0 + k) * D:(h0 + k + 1) * D] for k in range(HPC)]
        wq_rep = np.concatenate([wq_h[0], wq_h[0], wq_h[1], wq_h[1]], axis=1)
        bq_h = [bq[(h0 + k) * D:(h0 + k + 1) * D] for k in range(HPC)]
        bq4 = np.concatenate([bq_h[0], bq_h[0], bq_h[1], bq_h[1]]) * sc
        # one packed [128, 642] bf16 buffer: wq s0|s1, wk s0|s1, wv s0|s1
        # (each [128, m] slab), then bq4/bvb as raw f32 bytes
        slabs = []
        for w in (wq_rep, Wk[:, sl], Wv[:, sl]):
            wb = w.astype(ml_dtypes.bfloat16).view(np.uint16)
            slabs += [wb[0:128, :], wb[128:256, :]]
        slabs.append(np.ascontiguousarray(
            bq4.reshape(128, 1).astype(np.float32)).view(np.uint16))
        slabs.append(np.ascontiguousarray(
            np.tile(bv[sl][None, :], (128, 1)).astype(np.float32))
            .view(np.uint16))
        wpk = np.ascontiguousarray(
            np.concatenate(slabs, axis=1)).view(ml_dtypes.bfloat16)
        in_maps.append({
            "hT": hT[b],
            "Ab": Ab[b],
            "wpack": wpk,
        })

    nc = _build_bass()
    res = run_bass_kernel_spmd(
        nc, in_maps, core_ids=list(range(NCORES)),
        trace=os.environ.get("BASS_TRACE", "0") == "1",
    )
    LAST_RESULTS = res

    out = np.empty((B, HEADS, N, D), np.float32)
    for c in range(NCORES):
        b = c // CORES_PER_B
        h0 = HPC * (c % CORES_PER_B)
        oo = res.results[c]["o"]                  # [128, N] f32
        for hh in range(HPC):
            num = oo[hh * D:(hh + 1) * D, :]      # [32, N] unnormalized out^T
            den = oo[64 + 32 * hh, :]             # [N]
            out[b, h0 + hh] = (num / den[None, :]).T
    return out



# revision 47
# speedup vs baseline: 1.0501x; 1.0098x over previous
"""Adjacency-aware multi-head attention on 8 trn2 NeuronCores.

Math (per b, head k):
  Q = h[b] @ Wq[:, k] + bq[k]           [N, D]
  S[i, j] = (Q_i . K_j) / sqrt(D)
  P[j, i] = exp(S[i, j]) / sum_j exp(S[i, j])      (softmax over keys j)
  out[i, d] = sum_j P[j, i] * A[b, j, i] * V[j, d]

The K bias cancels: it adds g[i] = Q_i . bk to every score of query i,
and softmax over j is invariant to per-i shifts -> bk is dropped.

Sharding: 16 (b, head) pairs over 8 cores, 2 heads of the SAME b per core so
the A[b] stream is shared by both heads.

Device dataflow ([j, i] layout so A needs no transpose).  exp of all scores
(8.4M elem/core) is the dominant elementwise cost; it is SPLIT between the
ACT engine (exact exp) and the DVE (Schraudolph int16-bitcast affine approx,
runs at DVE 2x mode: ~0.58ns/col) so both engines land at ~58us busy:
  - Strip mapping r = 2*head + (t%2): j-tile t of head hh computes on PE row
    strip r.  Q^T is written strip-replicated directly by the Q projection
    (host passes Wq with columns [h0|h0|h1|h1]), no SBUF->SBUF copies.
    The K bias is dropped (cancels in softmax).
  - Slot structure: one slot per j-tile pair p (4 S blocks).  ACT pairs'
    blocks go through a 2-buf 2-bank "ps" PSUM tag consumed ONLY by ACT;
    Schraudolph pairs' blocks 4p+2,4p+3 go through a 1-buf "psv" tag whose
    PE fill is deferred one slot, so the saturated DVE queue can lag ~3
    slots without ever stalling the ACT exp rotation (coupling the two
    rotations was measured to cost ~10us in boundary stalls).
  - The pair's merged EA = E * A tensor_tensor (one 4D-AP op, 2x bf16)
    follows its Schraudolph in the same DVE queue -> no cross-engine wait.
  - Startup: each dma_start costs ~620ns of SERIAL sync-sequencer issue and
    per-queue rings are FIFO in posting order, so inputs are packed into 7
    fat DMAs in need-order (hT pieces are single 2KB runs per partition via
    a column-blocked host layout; all weights/biases in one "wpack" buffer;
    chunk-0 A split in two halves).  A dummy exp pulls the ~1.3us
    ACT_TABLE_LOAD into the preamble.  A few dummy matmuls warm the PE
    clock gate (kept small: a dense burst across 8 cores trips the
    chip-wide power throttle, which also makes run-to-run timing vary
    by ~10-20%).
  - phase 2 quads trail ~2 slots behind, 4 column-tiled streams into one
    PSUM tile: out_h0 (rows 0-31), out_h1 (32-63), denom_h0 (row 64),
    denom_h1 (row 96); late quads spill into the next chunk's slots 0-2.
    PSUM->SBUF output eviction runs on ACT (Copy) to keep DVE clear.
Device returns [128, N]: rows 0-31 outT_h0, 32-63 outT_h1, rows 64/96 the
softmax denominators.  Host does out = (outT / denom)^T plus the gather.

BASS_SCHR picks the per-chunk Schraudolph pairs (default 4;4;4;3 of 8 --
~23% of elements approximated, end-to-end rel err ~4.3e-3 vs 3.4e-3 exact).
"""

import math
import os

import numpy as np
import ml_dtypes

B, N, IN_DIM = 2, 2048, 256
HEADS, D = 8, 32
NCORES = 8
HPC = 2              # heads per core
NJ = N // 128        # 16 j-tiles
NCH = 4              # i-chunks
CH = N // NCH        # 512
CORES_PER_B = NCORES // B
GRP = 3              # S psum banks per exp op
NBLK = NJ * HPC      # 32 S blocks per chunk
NGRP = (NBLK + GRP - 1) // GRP   # 11 exp groups per chunk
NPAIR = NJ // 2      # 8 j-tile pairs

LAST_RESULTS = None  # BassKernelResults of the most recent kernel() call


def _build_bass():
    import concourse.bass as bass
    import concourse.mybir as mybir
    import concourse.tile as tile
    from concourse import bacc

    f32 = mybir.dt.float32
    bf16 = mybir.dt.bfloat16
    i16 = mybir.dt.int16
    AF = mybir.ActivationFunctionType

    # Schraudolph exp: bf16 bit pattern of exp(x) ~ int16(x*128*log2(e) +
    # 127*128 - c).  Piecewise-linear 2^frac approx, max rel err ~3.4%;
    # softmax normalization cancels most of it (numpy sim of the full
    # pipeline: ~1.0e-2 end-to-end at 25% of groups approximated, vs the
    # 2e-2 gate).  Offloads ACT -> DVE.
    SCH_A = 128 * 1.4426950408889634
    SCH_B = 127.0 * 128 - 7.0
    # per-chunk exp groups computed on DVE via Schraudolph instead of ACT
    # per-chunk ODD pairs whose blocks 4p+2,4p+3 are Schraudolphed on DVE
    # via a dedicated 1-buf "psv" PSUM tag (deferred fill) so the ACT exp
    # rotation never waits on the saturated DVE queue; last chunk keeps
    # pair 7 on ACT so the drain isn't DVE-gated
    _schr = os.environ.get("BASS_SCHR", "1,3,5,7;1,3,5,7;1,3,5,7;1,3,5")
    SCHRAUD = [tuple(int(x) for x in part.split(",") if x != "")
               for part in _schr.split(";")]
    # NOTE: offloading EA tensor_tensors to GPSIMD was measured
    # NET-NEGATIVE on hardware: GPSIMD streams through the DVE's second
    # SBUF port, and concurrent GPS TTs degrade DVE tensor_tensor from
    # ~1133ns to ~1430-2731ns per op (DVE lost ~23us to save 13us).

    nc = bacc.Bacc("TRN2", target_bir_lowering=False, debug=False,
                   num_devices=NCORES)

    # hT and A arrive host-relaid so each SBUF partition's bytes are one
    # contiguous DRAM run (8KB descriptors instead of 1-2KB: 16x fewer
    # descriptors -> faster SWDGE descgen and lower DMA-queue occupancy)
    hT = nc.dram_tensor("hT", [128, 2 * N], bf16, kind="ExternalInput").ap()
    Ab = nc.dram_tensor("Ab", [NCH * 128, NJ * CH], bf16,
                        kind="ExternalInput").ap()
    # all weights/biases packed in one buffer: each dma_start costs ~620ns
    # of serial sync-sequencer issue regardless of size, so one fat DMA
    # beats five thin ones.  bf16 cols: wq[s0|s1] 0:256, wk 256:384,
    # wv 384:512, bq4(f32) 512:514, bvb(f32) 514:642.
    WPK = 4 * D * 2 + HPC * D * 2 * 2 + 2 + HPC * D * 2
    wpack = nc.dram_tensor("wpack", [128, WPK], bf16,
                           kind="ExternalInput").ap()
    o = nc.dram_tensor("o", [128, N], f32, kind="ExternalOutput").ap()

    SC = 1.0 / math.sqrt(D)

    def bcast_free(ap_col, n):
        return bass.AP(tensor=ap_col.tensor, offset=ap_col.offset,
                       ap=[ap_col.ap[0], [0, n]])

    # block index for (head hh, j-tile t): strip r = 2*hh + t%2
    def blk(hh, t):
        return 4 * (t // 2) + 2 * hh + (t % 2)

    with (
        tile.TileContext(nc) as tc,
        tc.tile_pool(name="const", bufs=1) as cpool,
        tc.tile_pool(name="ps", bufs=2, space="PSUM") as pspool,
        tc.tile_pool(name="pod", bufs=2, space="PSUM") as podpool,
        tc.tile_pool(name="apool", bufs=2) as apool,
        tc.tile_pool(name="epool", bufs=2) as epool,
        tc.tile_pool(name="eapool", bufs=2) as eapool,
        tc.tile_pool(name="opool", bufs=2) as opool,
    ):
        # ---- constants / inputs into SBUF
        scratch = cpool.tile([128, CH], bf16, tag="scratch")
        nc.vector.memset(scratch, 0.0)
        ones_sb = cpool.tile([128, 1], bf16, tag="ones")
        nc.gpsimd.memset(ones_sb, 1.0)
        # dummy activation to pull the ~1.3us exp ACT_TABLE_LOAD into the
        # preamble instead of serializing before the first real exp
        warm_sb = cpool.tile([128, 1], bf16, tag="warm")
        nc.scalar.activation(warm_sb, ones_sb, AF.Exp)

        # startup DMAs in need-order (per-queue rings are FIFO in posting
        # order, so later DMAs queue behind earlier ones): Q/K-proj columns
        # of hT, the packed weights, the rest of hT half 0, the hT half-1
        # head for K-proj c1, then the chunk-0 A tile in two halves so the
        # early pairs' EA can start while the second half streams.
        # hT DRAM layout is column-blocked: [p, q(4), s(2), 512] so each
        # 512-col piece is ONE 2KB run per partition (128 fat descriptors)
        hT4 = hT.rearrange("p (q s n) -> p q s n", q=4, s=2)
        hT_half = [cpool.tile([128, 2, N // 2], bf16, tag=f"hT{c}",
                              name=f"hT{c}")
                   for c in range(2)]
        wpk_sb = cpool.tile([128, WPK], bf16, tag="wpk")
        nc.sync.dma_start(hT_half[0][:, :, 0:CH], hT4[:, 0])
        nc.sync.dma_start(wpk_sb, wpack)
        nc.sync.dma_start(hT_half[0][:, :, CH:2 * CH], hT4[:, 1])
        nc.sync.dma_start(hT_half[1][:, :, 0:CH], hT4[:, 2])

        _woff = {"q": (0, 4 * D), "k": (2 * 4 * D, HPC * D),
                 "v": (2 * 4 * D + 2 * HPC * D, HPC * D)}

        def wcols(name, s):
            off, m = _woff[name]
            return wpk_sb[:, off + s * m:off + (s + 1) * m]

        bq4_sb = wpk_sb[:, 512:514].bitcast(f32)
        bvb_sb = wpk_sb[:, 514:642].bitcast(f32)

        def hT_sb_cols(s, lo, hi):       # [lo, hi) within one half
            c = lo // (N // 2)
            assert (hi - 1) // (N // 2) == c
            return hT_half[c][:, s, lo - c * N // 2:hi - c * N // 2]

        qt4 = cpool.tile([128, N], bf16, tag="qt4")      # strips [h0|h0|h1|h1]
        kt4 = cpool.tile([128, NJ // 2 * 128], bf16, tag="kt4")
        Vt = cpool.tile([128, NJ * HPC * D], bf16, tag="vt")   # col = t*64+d

        A3 = Ab.rearrange("(c p) (t i) -> c p t i", p=128, i=CH)
        a_tiles = [None] * NCH

        def emit_a_dma(ch):
            a_t = apool.tile([128, NJ, CH], bf16, tag="a")
            nc.sync.dma_start(a_t, A3[ch])
            a_tiles[ch] = a_t

        # chunk-0 A in two halves bracketing the hT half-1 tail so the
        # K-proj c1 / late V-proj columns aren't stuck behind 2.1MB of A
        a0_t = apool.tile([128, NJ, CH], bf16, tag="a")
        nc.sync.dma_start(hT_half[1][:, :, CH:2 * CH], hT4[:, 3])
        nc.sync.dma_start(a0_t[:, 0:NJ // 2, :], A3[0][:, 0:NJ // 2])
        nc.sync.dma_start(a0_t[:, NJ // 2:, :], A3[0][:, NJ // 2:])
        a_tiles[0] = a0_t

        # ---- PE warm-up: a few dummy matmuls while input DMAs are in
        #      flight (kept small: a dense burst across all 8 cores trips
        #      the chip-wide power throttle and downclocks everything 1.2x)
        for _ in range(3):
            jt = podpool.tile([128, CH], f32, tag="od", name="junk")
            nc.tensor.matmul(jt, lhsT=scratch[:, 0:128], rhs=scratch,
                             start=True, stop=True)

        # ---- V projection helpers (all 16 tiles run inline in chunk 0)
        def emit_vproj_mms(vps, base, t0, cnt):
            for t in range(t0, t0 + cnt):
                for s in range(2):
                    nc.tensor.matmul(
                        vps[:, (t - base) * HPC * D:(t - base + 1) * HPC * D],
                        lhsT=hT_sb_cols(s, t * 128, (t + 1) * 128),
                        rhs=wcols("v", s),
                        start=(s == 0), stop=(s == 1),
                    )

        def emit_vproj_add(vps, t0, cnt=8):
            vsl = vps[:, (t0 % 8) * HPC * D:(t0 % 8 + cnt) * HPC * D]
            base = Vt[:, t0 * HPC * D:(t0 + cnt) * HPC * D]
            out_ap = bass.AP(tensor=base.tensor, offset=base.offset,
                             ap=[base.ap[0], [HPC * D, cnt], [1, HPC * D]])
            in_ap = bass.AP(tensor=vsl.tensor, offset=vsl.offset,
                            ap=[vsl.ap[0], [HPC * D, cnt], [1, HPC * D]])
            b_ap = bass.AP(tensor=bvb_sb.tensor, offset=bvb_sb.offset,
                           ap=[bvb_sb.ap[0], [0, cnt], [1, HPC * D]])
            nc.vector.tensor_add(out_ap, in_ap, b_ap)

        # ---- K projection into packed strip layout.
        #      strip r holds K^T of head r//2 for tiles t = 2*q2 + r%2.
        #      c=0 (j-tiles 0-7 -> S groups 0-4) runs on the first hT half
        #      before the loop; c=1 is emitted inside chunk 0, group 0.
        def emit_kproj(c):
            kps = pspool.tile([128, CH], f32, tag="ps", name=f"kps{c}")
            for r in range(4):
                for s in range(2):
                    base = hT_half[c][:, s, (r % 2) * 128:(r % 2) * 128 + 128]
                    rhs = bass.AP(tensor=base.tensor, offset=base.offset,
                                  ap=[base.ap[0], [256, 4], [1, 128]])
                    nc.tensor.matmul(
                        kps[32 * r:32 * r + 32, :],
                        lhsT=wcols("k", s)[:, (r // 2) * D:(r // 2 + 1) * D],
                        rhs=rhs,
                        start=(s == 0), stop=(s == 1),
                        tile_position=(0, 32 * r),
                    )
            return kps

        # bk dropped (cancels in softmax); cast on DVE so it runs in
        # parallel with the Q-proj eviction on ACT (startup critical path)
        kps0 = emit_kproj(0)
        nc.vector.tensor_copy(kt4[:, 0:CH], kps0[:, 0:CH])

        # ---- Q projection (+bias, scaled 1/sqrt(D)), strip-replicated
        #      directly: wq columns are [h0|h0|h1|h1] (host-packed).
        def emit_qproj(quarter, on_act=False):
            sl = slice(quarter * CH, (quarter + 1) * CH)
            qps = pspool.tile([128, CH], f32, tag="ps", name="qps")
            for s in range(2):
                nc.tensor.matmul(qps, lhsT=wcols("q", s),
                                 rhs=hT_sb_cols(s, quarter * CH,
                                                (quarter + 1) * CH),
                                 start=(s == 0), stop=(s == 1))
            if on_act:
                nc.scalar.activation(qt4[:, sl], qps, AF.Identity,
                                     bias=bq4_sb, scale=SC)
            else:
                nc.vector.scalar_tensor_tensor(
                    qt4[:, sl], qps, SC, bcast_free(bq4_sb, CH),
                    op0=mybir.AluOpType.mult, op1=mybir.AluOpType.add,
                )

        emit_qproj(0, on_act=True)

        # ---- main loop helpers
        def emit_amult(e_t, ea_t, a_t, p, hh):
            engine = nc.vector
            if hh is None:   # both heads of pair p in one op (4D AP)
                b0 = 4 * p * CH
                eb = e_t[:, b0:b0 + CH]
                e_ap = bass.AP(tensor=eb.tensor, offset=eb.offset,
                               ap=[eb.ap[0], [2 * CH, 2], [CH, 2], [1, CH]])
                eab = ea_t[:, b0:b0 + CH]
                ea_ap = bass.AP(tensor=eab.tensor, offset=eab.offset,
                                ap=[eab.ap[0], [2 * CH, 2], [CH, 2], [1, CH]])
                ab = a_t[:, 2 * p, :]
                a_ap = bass.AP(tensor=ab.tensor, offset=ab.offset,
                               ap=[ab.ap[0], [0, 2], [CH, 2], [1, CH]])
                engine.tensor_mul(ea_ap, e_ap, a_ap)
                return
            # blocks 4p+2hh, 4p+2hh+1 = head hh, j-tiles 2p, 2p+1: contiguous
            b0 = (4 * p + 2 * hh) * CH
            eb = e_t[:, b0:b0 + CH]
            e_ap = bass.AP(tensor=eb.tensor, offset=eb.offset,
                           ap=[eb.ap[0], [CH, 2], [1, CH]])
            eab = ea_t[:, b0:b0 + CH]
            ea_ap = bass.AP(tensor=eab.tensor, offset=eab.offset,
                            ap=[eab.ap[0], [CH, 2], [1, CH]])
            engine.tensor_mul(ea_ap, e_ap, a_t[:, 2 * p:2 * p + 2, :])

        def emit_ph2_quad(od, e_t, ea_t, p, hh, den_first=False):
            # NOTE: splitting these 128-contraction matmuls into 4 x 32-row
            # bands at tile positions (32r, col) -- to overlap like the S
            # strips do -- fails at runtime (INTERNAL error), so the
            # full-contraction form stays.
            def den(tp):
                t = 2 * p + tp
                bsl = slice(blk(hh, t) * CH, (blk(hh, t) + 1) * CH)
                nc.tensor.matmul(
                    od[64 + 32 * hh:65 + 32 * hh, :],
                    lhsT=ones_sb, rhs=e_t[:, bsl],
                    start=(t == 0), stop=(t == NJ - 1),
                    tile_position=(0, 64 + 32 * hh),
                )

            def vmm(tp):
                t = 2 * p + tp
                bsl = slice(blk(hh, t) * CH, (blk(hh, t) + 1) * CH)
                nc.tensor.matmul(
                    od[32 * hh:32 * hh + 32, :],
                    lhsT=Vt[:, t * 64 + 32 * hh:t * 64 + 32 * hh + 32],
                    rhs=ea_t[:, bsl],
                    start=(t == 0), stop=(t == NJ - 1),
                    tile_position=(0, 32 * hh),
                )

            if den_first:   # dens only need E, not the DVE product
                den(0), den(1), vmm(0), vmm(1)
            else:
                vmm(0), den(0), vmm(1), den(1)

        def emit_out(od, ch):
            # PSUM->SBUF eviction on ACT (measured better than DVE here);
            # DMA cannot read PSUM directly.
            o_sb = opool.tile([128, CH], f32, tag="o")
            nc.scalar.activation(o_sb, od, AF.Copy)
            nc.sync.dma_start(o[:, ch * CH:(ch + 1) * CH], o_sb)

        carry = None   # (od, e_t, ea_t, ch, quads) spill of previous chunk
        for ch in range(NCH):
            if ch + 1 < NCH:
                emit_a_dma(ch + 1)
            a_t = a_tiles[ch]
            e_t = epool.tile([128, NBLK * CH], bf16, tag="e")
            ea_t = eapool.tile([128, NBLK * CH], bf16, tag="ea")
            od = None

            # slot schedule: one slot per pair p.  Each slot computes the
            # pair's 4 S blocks: blocks 4p,4p+1 into a 2-bank "ps" tile and
            # 4p+2(,4p+3) into another, all consumed by ACT exp; for
            # Schraudolph pairs block 4p+3 goes to a separate 1-bank "psv"
            # tile consumed by DVE, so the two PSUM rotations never couple
            # the ACT exp stream to the saturated DVE queue.  The pair's
            # merged EA tensor_tensor follows its Schraudolph in the same
            # DVE queue; phase2 quads trail by l0 slots and spill into the
            # next chunk's slots 0-2.
            last = ch == NCH - 1
            sch_pairs = SCHRAUD[ch] if ch < len(SCHRAUD) else ()
            amult_at = {}
            epilogue_amults = []
            ph2_at = {}
            spill = []

            # od accumulation (start at t=0) requires quads of one head to
            # hit the PE queue in j-tile order: clamp to the head's running
            # max slot (append order within a slot is p-ascending).
            last_gq = [0, 0]

            def sched_quad(gq, p, hh):
                gq = max(gq, last_gq[hh])
                last_gq[hh] = gq
                if gq < NPAIR:
                    ph2_at.setdefault(gq, []).append((p, hh))
                else:
                    spill.append((p, hh))

            for p in range(NPAIR):
                schr = p in sch_pairs
                base_slot = p + 1 if schr else p
                if ch == 0:
                    # chunk-0 EA waits the A0 halves landing (~slots 2/4)
                    base_slot = max(base_slot, 2 if p < 4 else p)
                if last and p >= 6:      # tail: per-head DVE TTs, min lag
                    for hh in range(HPC):
                        amult_at.setdefault(p, []).append((p, hh))
                        sched_quad(min(p + 1, NPAIR - 1), p, hh)
                else:
                    if base_slot < NPAIR:
                        amult_at.setdefault(base_slot, []).append((p, None))
                    else:
                        epilogue_amults.append((p, None))
                    lag = 2 if last else 4
                    for hh in range(HPC):
                        sched_quad(base_slot + lag + hh, p, hh)

            def emit_s(dest, b):
                q2, r = b // 4, b % 4
                nc.tensor.matmul(
                    dest,
                    lhsT=kt4[32 * r:32 * r + 32, q2 * 128:(q2 + 1) * 128],
                    rhs=qt4[32 * r:32 * r + 32, ch * CH:(ch + 1) * CH],
                    start=True, stop=True,
                    tile_position=(32 * r, 0),
                )

            def emit_schr_pair(pp):
                # 1-buf psv tile, PE-fill deferred one slot: schraud(pp-2)
                # has ~3 slots to run before this fill waits on it
                tv = pspool.tile([128, 2 * CH], f32, tag="psv", bufs=1,
                                 name="sv")
                emit_s(tv[:, 0:CH], 4 * pp + 2)
                emit_s(tv[:, CH:], 4 * pp + 3)
                nc.vector.tensor_scalar(
                    e_t[:, (4 * pp + 2) * CH:(4 * pp + 4) * CH].bitcast(i16),
                    tv, SCH_A, SCH_B,
                    op0=mybir.AluOpType.mult, op1=mybir.AluOpType.add)

            pending_schr = None
            for p in range(NPAIR):
                b0 = 4 * p
                schr = p in sch_pairs
                ta = pspool.tile([128, 2 * CH], f32, tag="ps", name="sA")
                emit_s(ta[:, 0:CH], b0)
                emit_s(ta[:, CH:], b0 + 1)
                if not schr:
                    tb = pspool.tile([128, 2 * CH], f32, tag="ps", name="sB")
                    emit_s(tb[:, 0:CH], b0 + 2)
                    emit_s(tb[:, CH:], b0 + 3)
                if pending_schr is not None:
                    emit_schr_pair(pending_schr)
                    pending_schr = None
                if schr:
                    pending_schr = p

                nc.scalar.activation(e_t[:, b0 * CH:(b0 + 2) * CH], ta,
                                     AF.Exp)
                if not schr:
                    nc.scalar.activation(
                        e_t[:, (b0 + 2) * CH:(b0 + 4) * CH], tb, AF.Exp)

                for pp, hh in amult_at.get(p, ()):
                    emit_amult(e_t, ea_t, a_t, pp, hh)

                # drain previous chunk's spilled phase2 quads: 2 per slot,
                # finishing at slot 4 (10 spill quads with lag 4)
                if carry is not None and p <= 4:
                    cod, ce, cea, cch, cquads = carry
                    take = cquads[:2] if p < 4 else cquads
                    for pp, hh in take:
                        emit_ph2_quad(cod, ce, cea, pp, hh)
                    cquads = cquads[len(take):]
                    if not cquads:
                        emit_out(cod, cch)
                        carry = None
                    else:
                        carry = (cod, ce, cea, cch, cquads)

                if ch == 0:
                    if p == 1:
                        # K projection second half at slot 1 (hT half 1
                        # lands ~slot 0.5; emitting at slot 0 head-of-line
                        # blocks the PE queue on the DMA); cast on ACT so
                        # the "ps" rotation's consumers stay single-engine
                        kps1 = emit_kproj(1)
                        nc.scalar.activation(kt4[:, CH:2 * CH], kps1,
                                             AF.Copy)
                    if p < 4:
                        if p == 0:
                            vps0 = podpool.tile([128, CH], f32, tag="od",
                                                name="vps0")
                        emit_vproj_mms(vps0, 0, 2 * p, 2)
                        if p == 1:
                            emit_vproj_add(vps0, 0, 4)
                        elif p == 3:
                            emit_vproj_add(vps0, 4, 4)
                    else:
                        if p == 4:
                            vps1 = podpool.tile([128, CH], f32, tag="od",
                                                name="vps1")
                        emit_vproj_mms(vps1, 8, 2 * p, 2)
                        if p == 5:
                            emit_vproj_add(vps1, 8, 4)
                        elif p == 7:
                            emit_vproj_add(vps1, 12, 4)
                    if p == 5:
                        emit_qproj(1, on_act=True)
                elif ch in (1, 2) and p == 5:
                    emit_qproj(ch + 1, on_act=True)

                for pp, hh in ph2_at.get(p, ()):
                    if od is None:
                        od = podpool.tile([128, CH], f32, tag="od")
                    emit_ph2_quad(od, e_t, ea_t, pp, hh)

            # chunk epilogue: pair-7's deferred Schraudolph + EA
            if pending_schr is not None:
                emit_schr_pair(pending_schr)
                pending_schr = None
            for pp, hh in epilogue_amults:
                emit_amult(e_t, ea_t, a_t, pp, hh)

            carry = (od, e_t, ea_t, ch, spill)

        cod, ce, cea, cch, cquads = carry
        for p, hh in cquads:
            emit_ph2_quad(cod, ce, cea, p, hh)
        emit_out(cod, cch)

    nc.finalize()
    return nc


def kernel(h, A, Wq, bq, Wk, bk, Wv, bv):
    global LAST_RESULTS
    from concourse.bass_utils import run_bass_kernel_spmd

    h = np.asarray(h, np.float32)
    A = np.asarray(A, np.float32)
    Wq = np.asarray(Wq, np.float32)
    Wk = np.asarray(Wk, np.float32)
    Wv = np.asarray(Wv, np.float32)
    bq = np.asarray(bq, np.float32)
    bv = np.asarray(bv, np.float32)

    # hT: [b, 128(p), q(4), s(2), 512] column-blocked so each 512-col DMA
    # piece is one contiguous 2KB run per partition
    hT = (h.transpose(0, 2, 1)                     # [b, IN_DIM, N]
          .reshape(B, 2, 128, N).transpose(0, 2, 1, 3)
          .reshape(B, 128, 2, 4, 512).transpose(0, 1, 3, 2, 4)
          .reshape(B, 128, 2 * N))
    hT = np.ascontiguousarray(hT).astype(ml_dtypes.bfloat16)
    # A: [b, ch*128(p), t*CH(i)]: a_t[p, t, i] = A[b, t*128+p, ch*CH+i]
    Ab = (A.reshape(B, NJ, 128, NCH, CH).transpose(0, 3, 2, 1, 4)
          .reshape(B, NCH * 128, NJ * CH))
    Ab = np.ascontiguousarray(Ab).astype(ml_dtypes.bfloat16)
    sc = np.float32(1.0 / math.sqrt(D))

    in_maps = []
    for c in range(NCORES):
        b = c // CORES_PER_B
        h0 = HPC * (c % CORES_PER_B)
        sl = slice(h0 * D, (h0 + HPC) * D)
        wq_h = [Wq[:, (h0 + k) * D:(h0 + k + 1) * D] for k in range(HPC)]
        wq_rep = np.concatenate([wq_h[0], wq_h[0], wq_h[1], wq_h[1]], axis=1)
        bq_h = [bq[(h0 + k) * D:(h0 + k + 1) * D] for k in range(HPC)]
        bq4 = np.concatenate([bq_h[0], bq_h[0], bq_h[1], bq_h[1]]) * sc
        # one packed [128, 642] bf16 buffer: wq s0|s1, wk s0|s1, wv s0|s1
        # (each [128, m] slab), then bq4/bvb as raw f32 bytes
        slabs = []
        for w in (wq_rep, Wk[:, sl], Wv[:, sl]):
            wb = w.astype(ml_dtypes.bfloat16).view(np.uint16)
            slabs += [wb[0:128, :], wb[128:256, :]]
        slabs.append(np.ascontiguousarray(
            bq4.reshape(128, 1).astype(np.float32)).view(np.uint16))
        slabs.append(np.ascontiguousarray(
            np.tile(bv[sl][None, :], (128, 1)).astype(np.float32))
            .view(np.uint16))
        wpk = np.ascontiguousarray(
            np.concatenate(slabs, axis=1)).view(ml_dtypes.bfloat16)
        in_maps.append({
            "hT": hT[b],
            "Ab": Ab[b],
            "wpack": wpk,
        })

    nc = _build_bass()
    res = run_bass_kernel_spmd(
        nc, in_maps, core_ids=list(range(NCORES)),
        trace=os.environ.get("BASS_TRACE", "0") == "1",
    )
    LAST_RESULTS = res

    out = np.empty((B, HEADS, N, D), np.float32)
    for c in range(NCORES):
        b = c // CORES_PER_B
        h0 = HPC * (c % CORES_PER_B)
        oo = res.results[c]["o"]                  # [128, N] f32
        for hh in range(HPC):
            num = oo[hh * D:(hh + 1) * D, :]      # [32, N] unnormalized out^T
            den = oo[64 + 32 * hh, :]             # [N]
            out[b, h0 + hh] = (num / den[None, :]).T
    return out



# revision 48
# speedup vs baseline: 1.0840x; 1.0323x over previous
"""Adjacency-aware multi-head attention on 8 trn2 NeuronCores.

Math (per b, head k):
  Q = h[b] @ Wq[:, k] + bq[k]           [N, D]
  S[i, j] = (Q_i . K_j) / sqrt(D)
  P[j, i] = exp(S[i, j]) / sum_j exp(S[i, j])      (softmax over keys j)
  out[i, d] = sum_j P[j, i] * A[b, j, i] * V[j, d]

The K bias cancels: it adds g[i] = Q_i . bk to every score of query i,
and softmax over j is invariant to per-i shifts -> bk is dropped.

Sharding: 16 (b, head) pairs over 8 cores, 2 heads of the SAME b per core so
the A[b] stream is shared by both heads.

Device dataflow ([j, i] layout so A needs no transpose).  exp of all scores
(8.4M elem/core) is the dominant elementwise cost; it is SPLIT between the
ACT engine (exact exp) and the DVE (Schraudolph int16-bitcast affine approx,
runs at DVE 2x mode: ~0.58ns/col) so both engines land at ~58us busy:
  - Strip mapping r = 2*head + (t%2): j-tile t of head hh computes on PE row
    strip r.  Q^T is written strip-replicated directly by the Q projection
    (host passes Wq with columns [h0|h0|h1|h1]), no SBUF->SBUF copies.
    The K bias is dropped (cancels in softmax).
  - Slot structure: one slot per j-tile pair p (4 S blocks).  ACT pairs'
    blocks go through a 2-buf 2-bank "ps" PSUM tag consumed ONLY by ACT;
    Schraudolph pairs' blocks 4p+2,4p+3 go through a 1-buf "psv" tag whose
    PE fill is deferred one slot, so the saturated DVE queue can lag ~3
    slots without ever stalling the ACT exp rotation (coupling the two
    rotations was measured to cost ~10us in boundary stalls).
  - The pair's merged EA = E * A tensor_tensor (one 4D-AP op, 2x bf16)
    follows its Schraudolph in the same DVE queue -> no cross-engine wait.
  - Startup: each dma_start costs ~620ns of SERIAL sync-sequencer issue and
    per-queue rings are FIFO in posting order, so inputs are packed into 7
    fat DMAs in need-order (hT pieces are single 2KB runs per partition via
    a column-blocked host layout; all weights/biases in one "wpack" buffer;
    chunk-0 A split in two halves).  A dummy exp pulls the ~1.3us
    ACT_TABLE_LOAD into the preamble.  A few dummy matmuls warm the PE
    clock gate (kept small: a dense burst across 8 cores trips the
    chip-wide power throttle, which also makes run-to-run timing vary
    by ~10-20%).
  - phase 2 quads trail ~2 slots behind, 4 column-tiled streams into one
    PSUM tile: out_h0 (rows 0-31), out_h1 (32-63), denom_h0 (row 64),
    denom_h1 (row 96); late quads spill into the next chunk's slots 0-2.
    PSUM->SBUF output eviction runs on ACT (Copy) to keep DVE clear.
Device returns [128, N]: rows 0-31 outT_h0, 32-63 outT_h1, rows 64/96 the
softmax denominators.  Host does out = (outT / denom)^T plus the gather.

BASS_SCHR picks the per-chunk Schraudolph pairs (default 4;4;4;3 of 8 --
~23% of elements approximated, end-to-end rel err ~4.3e-3 vs 3.4e-3 exact).
"""

import math
import os

import numpy as np
import ml_dtypes

B, N, IN_DIM = 2, 2048, 256
HEADS, D = 8, 32
NCORES = 8
HPC = 2              # heads per core
NJ = N // 128        # 16 j-tiles
NCH = 4              # i-chunks
CH = N // NCH        # 512
CORES_PER_B = NCORES // B
GRP = 3              # S psum banks per exp op
NBLK = NJ * HPC      # 32 S blocks per chunk
NGRP = (NBLK + GRP - 1) // GRP   # 11 exp groups per chunk
NPAIR = NJ // 2      # 8 j-tile pairs

LAST_RESULTS = None  # BassKernelResults of the most recent kernel() call


def _build_bass():
    import concourse.bass as bass
    import concourse.mybir as mybir
    import concourse.tile as tile
    from concourse import bacc

    f32 = mybir.dt.float32
    bf16 = mybir.dt.bfloat16
    i16 = mybir.dt.int16
    AF = mybir.ActivationFunctionType

    # Schraudolph exp: bf16 bit pattern of exp(x) ~ int16(x*128*log2(e) +
    # 127*128 - c).  Piecewise-linear 2^frac approx, max rel err ~3.4%;
    # softmax normalization cancels most of it (numpy sim of the full
    # pipeline: ~1.0e-2 end-to-end at 25% of groups approximated, vs the
    # 2e-2 gate).  Offloads ACT -> DVE.
    SCH_A = 128 * 1.4426950408889634
    SCH_B = 127.0 * 128 - 7.0
    # per-chunk exp groups computed on DVE via Schraudolph instead of ACT
    # per-chunk ODD pairs whose blocks 4p+2,4p+3 are Schraudolphed on DVE
    # via a dedicated 1-buf "psv" PSUM tag (deferred fill) so the ACT exp
    # rotation never waits on the saturated DVE queue; last chunk keeps
    # pair 7 on ACT so the drain isn't DVE-gated
    _schr = os.environ.get("BASS_SCHR", "1,3,5,7;1,3,5,7;1,3,5,7;1,3,5")
    SCHRAUD = [tuple(int(x) for x in part.split(",") if x != "")
               for part in _schr.split(";")]
    # NOTE: offloading EA tensor_tensors to GPSIMD was measured
    # NET-NEGATIVE on hardware: GPSIMD streams through the DVE's second
    # SBUF port, and concurrent GPS TTs degrade DVE tensor_tensor from
    # ~1133ns to ~1430-2731ns per op (DVE lost ~23us to save 13us).

    nc = bacc.Bacc("TRN2", target_bir_lowering=False, debug=False,
                   num_devices=NCORES)

    # hT and A arrive host-relaid so each SBUF partition's bytes are one
    # contiguous DRAM run (8KB descriptors instead of 1-2KB: 16x fewer
    # descriptors -> faster SWDGE descgen and lower DMA-queue occupancy)
    hT = nc.dram_tensor("hT", [128, 2 * N], bf16, kind="ExternalInput").ap()
    Ab = nc.dram_tensor("Ab", [NCH * 128, NJ * CH], bf16,
                        kind="ExternalInput").ap()
    # all weights/biases packed in one buffer: each dma_start costs ~620ns
    # of serial sync-sequencer issue regardless of size, so one fat DMA
    # beats five thin ones.  bf16 cols: wq[s0|s1] 0:256, wk 256:384,
    # wv 384:512, bq4(f32) 512:514, bvb(f32) 514:642.
    WPK = 4 * D * 2 + HPC * D * 2 * 2 + 2 + HPC * D * 2
    wpack = nc.dram_tensor("wpack", [128, WPK], bf16,
                           kind="ExternalInput").ap()
    o = nc.dram_tensor("o", [128, N], f32, kind="ExternalOutput").ap()

    SC = 1.0 / math.sqrt(D)

    def bcast_free(ap_col, n):
        return bass.AP(tensor=ap_col.tensor, offset=ap_col.offset,
                       ap=[ap_col.ap[0], [0, n]])

    # block index for (head hh, j-tile t): strip r = 2*hh + t%2
    def blk(hh, t):
        return 4 * (t // 2) + 2 * hh + (t % 2)

    with (
        tile.TileContext(nc) as tc,
        tc.tile_pool(name="const", bufs=1) as cpool,
        tc.tile_pool(name="ps", bufs=2, space="PSUM") as pspool,
        tc.tile_pool(name="pod", bufs=2, space="PSUM") as podpool,
        tc.tile_pool(name="apool", bufs=2) as apool,
        tc.tile_pool(name="epool", bufs=2) as epool,
        tc.tile_pool(name="eapool", bufs=2) as eapool,
        tc.tile_pool(name="opool", bufs=2) as opool,
    ):
        # ---- constants / inputs into SBUF
        scratch = cpool.tile([128, CH], bf16, tag="scratch")
        nc.vector.memset(scratch, 0.0)
        ones_sb = cpool.tile([128, 1], bf16, tag="ones")
        nc.gpsimd.memset(ones_sb, 1.0)
        # dummy activation to pull the ~1.3us exp ACT_TABLE_LOAD into the
        # preamble instead of serializing before the first real exp
        warm_sb = cpool.tile([128, 1], bf16, tag="warm")
        nc.scalar.activation(warm_sb, ones_sb, AF.Exp)

        # startup DMAs in need-order (per-queue rings are FIFO in posting
        # order, so later DMAs queue behind earlier ones): Q/K-proj columns
        # of hT, the packed weights, the rest of hT half 0, the hT half-1
        # head for K-proj c1, then the chunk-0 A tile in two halves so the
        # early pairs' EA can start while the second half streams.
        # hT DRAM layout is column-blocked: [p, q(4), s(2), 512] so each
        # 512-col piece is ONE 2KB run per partition (128 fat descriptors)
        hT4 = hT.rearrange("p (q s n) -> p q s n", q=4, s=2)
        hT_half = [cpool.tile([128, 2, N // 2], bf16, tag=f"hT{c}",
                              name=f"hT{c}")
                   for c in range(2)]
        wpk_sb = cpool.tile([128, WPK], bf16, tag="wpk")
        nc.sync.dma_start(hT_half[0][:, :, 0:CH], hT4[:, 0])
        nc.sync.dma_start(wpk_sb, wpack)
        nc.sync.dma_start(hT_half[0][:, :, CH:2 * CH], hT4[:, 1])
        nc.sync.dma_start(hT_half[1][:, :, 0:CH], hT4[:, 2])

        _woff = {"q": (0, 4 * D), "k": (2 * 4 * D, HPC * D),
                 "v": (2 * 4 * D + 2 * HPC * D, HPC * D)}

        def wcols(name, s):
            off, m = _woff[name]
            return wpk_sb[:, off + s * m:off + (s + 1) * m]

        bq4_sb = wpk_sb[:, 512:514].bitcast(f32)
        bvb_sb = wpk_sb[:, 514:642].bitcast(f32)

        def hT_sb_cols(s, lo, hi):       # [lo, hi) within one half
            c = lo // (N // 2)
            assert (hi - 1) // (N // 2) == c
            return hT_half[c][:, s, lo - c * N // 2:hi - c * N // 2]

        qt4 = cpool.tile([128, N], bf16, tag="qt4")      # strips [h0|h0|h1|h1]
        kt4 = cpool.tile([128, NJ // 2 * 128], bf16, tag="kt4")
        Vt = cpool.tile([128, NJ * HPC * D], bf16, tag="vt")   # col = t*64+d

        A3 = Ab.rearrange("(c p) (t i) -> c p t i", p=128, i=CH)
        a_tiles = [None] * NCH

        def emit_a_dma(ch):
            a_t = apool.tile([128, NJ, CH], bf16, tag="a")
            nc.sync.dma_start(a_t, A3[ch])
            a_tiles[ch] = a_t

        # chunk-0 A in two halves bracketing the hT half-1 tail so the
        # K-proj c1 / late V-proj columns aren't stuck behind 2.1MB of A
        a0_t = apool.tile([128, NJ, CH], bf16, tag="a")
        nc.sync.dma_start(hT_half[1][:, :, CH:2 * CH], hT4[:, 3])
        nc.sync.dma_start(a0_t[:, 0:NJ // 2, :], A3[0][:, 0:NJ // 2])
        nc.sync.dma_start(a0_t[:, NJ // 2:, :], A3[0][:, NJ // 2:])
        a_tiles[0] = a0_t

        # ---- PE warm-up: a few dummy matmuls while input DMAs are in
        #      flight (kept small: a dense burst across all 8 cores trips
        #      the chip-wide power throttle and downclocks everything 1.2x)
        for _ in range(3):
            jt = podpool.tile([128, CH], f32, tag="od", name="junk")
            nc.tensor.matmul(jt, lhsT=scratch[:, 0:128], rhs=scratch,
                             start=True, stop=True)

        # ---- V projection helpers (all 16 tiles run inline in chunk 0)
        def emit_vproj_mms(vps, base, t0, cnt):
            for t in range(t0, t0 + cnt):
                for s in range(2):
                    nc.tensor.matmul(
                        vps[:, (t - base) * HPC * D:(t - base + 1) * HPC * D],
                        lhsT=hT_sb_cols(s, t * 128, (t + 1) * 128),
                        rhs=wcols("v", s),
                        start=(s == 0), stop=(s == 1),
                    )

        def emit_vproj_add(vps, t0, cnt=8):
            vsl = vps[:, (t0 % 8) * HPC * D:(t0 % 8 + cnt) * HPC * D]
            base = Vt[:, t0 * HPC * D:(t0 + cnt) * HPC * D]
            out_ap = bass.AP(tensor=base.tensor, offset=base.offset,
                             ap=[base.ap[0], [HPC * D, cnt], [1, HPC * D]])
            in_ap = bass.AP(tensor=vsl.tensor, offset=vsl.offset,
                            ap=[vsl.ap[0], [HPC * D, cnt], [1, HPC * D]])
            b_ap = bass.AP(tensor=bvb_sb.tensor, offset=bvb_sb.offset,
                           ap=[bvb_sb.ap[0], [0, cnt], [1, HPC * D]])
            nc.vector.tensor_add(out_ap, in_ap, b_ap)

        # ---- K projection into packed strip layout.
        #      strip r holds K^T of head r//2 for tiles t = 2*q2 + r%2.
        #      c=0 (j-tiles 0-7 -> S groups 0-4) runs on the first hT half
        #      before the loop; c=1 is emitted inside chunk 0, group 0.
        def emit_kproj(c):
            kps = pspool.tile([128, CH], f32, tag="ps", name=f"kps{c}")
            for r in range(4):
                for s in range(2):
                    base = hT_half[c][:, s, (r % 2) * 128:(r % 2) * 128 + 128]
                    rhs = bass.AP(tensor=base.tensor, offset=base.offset,
                                  ap=[base.ap[0], [256, 4], [1, 128]])
                    nc.tensor.matmul(
                        kps[32 * r:32 * r + 32, :],
                        lhsT=wcols("k", s)[:, (r // 2) * D:(r // 2 + 1) * D],
                        rhs=rhs,
                        start=(s == 0), stop=(s == 1),
                        tile_position=(0, 32 * r),
                    )
            return kps

        # bk dropped (cancels in softmax); cast on DVE so it runs in
        # parallel with the Q-proj eviction on ACT (startup critical path)
        kps0 = emit_kproj(0)
        nc.vector.tensor_copy(kt4[:, 0:CH], kps0[:, 0:CH])

        # ---- Q projection (+bias, scaled 1/sqrt(D)), strip-replicated
        #      directly: wq columns are [h0|h0|h1|h1] (host-packed).
        def emit_qproj(quarter, on_act=False):
            sl = slice(quarter * CH, (quarter + 1) * CH)
            qps = pspool.tile([128, CH], f32, tag="ps", name="qps")
            for s in range(2):
                nc.tensor.matmul(qps, lhsT=wcols("q", s),
                                 rhs=hT_sb_cols(s, quarter * CH,
                                                (quarter + 1) * CH),
                                 start=(s == 0), stop=(s == 1))
            if on_act:
                nc.scalar.activation(qt4[:, sl], qps, AF.Identity,
                                     bias=bq4_sb, scale=SC)
            else:
                nc.vector.scalar_tensor_tensor(
                    qt4[:, sl], qps, SC, bcast_free(bq4_sb, CH),
                    op0=mybir.AluOpType.mult, op1=mybir.AluOpType.add,
                )

        emit_qproj(0, on_act=True)

        # ---- main loop helpers
        def emit_amult(e_t, ea_t, a_t, p, hh):
            engine = nc.vector
            if hh is None:   # both heads of pair p in one op (4D AP)
                b0 = 4 * p * CH
                eb = e_t[:, b0:b0 + CH]
                e_ap = bass.AP(tensor=eb.tensor, offset=eb.offset,
                               ap=[eb.ap[0], [2 * CH, 2], [CH, 2], [1, CH]])
                eab = ea_t[:, b0:b0 + CH]
                ea_ap = bass.AP(tensor=eab.tensor, offset=eab.offset,
                                ap=[eab.ap[0], [2 * CH, 2], [CH, 2], [1, CH]])
                ab = a_t[:, 2 * p, :]
                a_ap = bass.AP(tensor=ab.tensor, offset=ab.offset,
                               ap=[ab.ap[0], [0, 2], [CH, 2], [1, CH]])
                engine.tensor_mul(ea_ap, e_ap, a_ap)
                return
            # blocks 4p+2hh, 4p+2hh+1 = head hh, j-tiles 2p, 2p+1: contiguous
            b0 = (4 * p + 2 * hh) * CH
            eb = e_t[:, b0:b0 + CH]
            e_ap = bass.AP(tensor=eb.tensor, offset=eb.offset,
                           ap=[eb.ap[0], [CH, 2], [1, CH]])
            eab = ea_t[:, b0:b0 + CH]
            ea_ap = bass.AP(tensor=eab.tensor, offset=eab.offset,
                            ap=[eab.ap[0], [CH, 2], [1, CH]])
            engine.tensor_mul(ea_ap, e_ap, a_t[:, 2 * p:2 * p + 2, :])

        def emit_ph2_quad(od, e_t, ea_t, p, hh, den_first=False):
            # NOTE: splitting these 128-contraction matmuls into 4 x 32-row
            # bands at tile positions (32r, col) -- to overlap like the S
            # strips do -- fails at runtime (INTERNAL error), so the
            # full-contraction form stays.
            def den(tp):
                t = 2 * p + tp
                bsl = slice(blk(hh, t) * CH, (blk(hh, t) + 1) * CH)
                nc.tensor.matmul(
                    od[64 + 32 * hh:65 + 32 * hh, :],
                    lhsT=ones_sb, rhs=e_t[:, bsl],
                    start=(t == 0), stop=(t == NJ - 1),
                    tile_position=(0, 64 + 32 * hh),
                )

            def vmm(tp):
                t = 2 * p + tp
                bsl = slice(blk(hh, t) * CH, (blk(hh, t) + 1) * CH)
                nc.tensor.matmul(
                    od[32 * hh:32 * hh + 32, :],
                    lhsT=Vt[:, t * 64 + 32 * hh:t * 64 + 32 * hh + 32],
                    rhs=ea_t[:, bsl],
                    start=(t == 0), stop=(t == NJ - 1),
                    tile_position=(0, 32 * hh),
                )

            if den_first:   # dens only need E, not the DVE product
                den(0), den(1), vmm(0), vmm(1)
            else:
                vmm(0), den(0), vmm(1), den(1)

        def emit_out(od, ch):
            # PSUM->SBUF eviction on ACT (measured better than DVE here);
            # DMA cannot read PSUM directly.
            o_sb = opool.tile([128, CH], f32, tag="o")
            nc.scalar.activation(o_sb, od, AF.Copy)
            nc.sync.dma_start(o[:, ch * CH:(ch + 1) * CH], o_sb)

        carry = None   # (od, e_t, ea_t, ch, quads) spill of previous chunk
        for ch in range(NCH):
            if ch + 1 < NCH:
                emit_a_dma(ch + 1)
            a_t = a_tiles[ch]
            e_t = epool.tile([128, NBLK * CH], bf16, tag="e")
            ea_t = eapool.tile([128, NBLK * CH], bf16, tag="ea")
            od = None

            # slot schedule: one slot per pair p.  Each slot computes the
            # pair's 4 S blocks: blocks 4p,4p+1 into a 2-bank "ps" tile and
            # 4p+2(,4p+3) into another, all consumed by ACT exp; for
            # Schraudolph pairs block 4p+3 goes to a separate 1-bank "psv"
            # tile consumed by DVE, so the two PSUM rotations never couple
            # the ACT exp stream to the saturated DVE queue.  The pair's
            # merged EA tensor_tensor follows its Schraudolph in the same
            # DVE queue; phase2 quads trail by l0 slots and spill into the
            # next chunk's slots 0-2.
            last = ch == NCH - 1
            sch_pairs = SCHRAUD[ch] if ch < len(SCHRAUD) else ()
            amult_at = {}
            epilogue_amults = []
            ph2_at = {}
            spill = []

            # od accumulation (start at t=0) requires quads of one head to
            # hit the PE queue in j-tile order: clamp to the head's running
            # max slot (append order within a slot is p-ascending).
            last_gq = [0, 0]

            def sched_quad(gq, p, hh):
                gq = max(gq, last_gq[hh])
                last_gq[hh] = gq
                if gq < NPAIR:
                    ph2_at.setdefault(gq, []).append((p, hh))
                else:
                    spill.append((p, hh))

            for p in range(NPAIR):
                schr = p in sch_pairs
                base_slot = p + 1 if schr else p
                if ch == 0:
                    # chunk-0 EA waits the A0 halves landing (~slots 2/4)
                    base_slot = max(base_slot, 2 if p < 4 else p)
                if last and p >= 6:      # tail: per-head DVE TTs, min lag
                    for hh in range(HPC):
                        amult_at.setdefault(p, []).append((p, hh))
                        sched_quad(min(p + 1, NPAIR - 1), p, hh)
                else:
                    if base_slot < NPAIR:
                        amult_at.setdefault(base_slot, []).append((p, None))
                    else:
                        epilogue_amults.append((p, None))
                    lag = 2 if last else 5
                    for hh in range(HPC):
                        sched_quad(base_slot + lag + hh, p, hh)

            def emit_s(dest, b):
                q2, r = b // 4, b % 4
                nc.tensor.matmul(
                    dest,
                    lhsT=kt4[32 * r:32 * r + 32, q2 * 128:(q2 + 1) * 128],
                    rhs=qt4[32 * r:32 * r + 32, ch * CH:(ch + 1) * CH],
                    start=True, stop=True,
                    tile_position=(32 * r, 0),
                )

            def emit_schr_pair(pp):
                # 1-buf psv tile, PE-fill deferred one slot: schraud(pp-2)
                # has ~3 slots to run before this fill waits on it
                tv = pspool.tile([128, 2 * CH], f32, tag="psv", bufs=1,
                                 name="sv")
                emit_s(tv[:, 0:CH], 4 * pp + 2)
                emit_s(tv[:, CH:], 4 * pp + 3)
                nc.vector.tensor_scalar(
                    e_t[:, (4 * pp + 2) * CH:(4 * pp + 4) * CH].bitcast(i16),
                    tv, SCH_A, SCH_B,
                    op0=mybir.AluOpType.mult, op1=mybir.AluOpType.add)

            pending_schr = None
            for p in range(NPAIR):
                b0 = 4 * p
                schr = p in sch_pairs
                ta = pspool.tile([128, 2 * CH], f32, tag="ps", name="sA")
                emit_s(ta[:, 0:CH], b0)
                emit_s(ta[:, CH:], b0 + 1)
                if not schr:
                    tb = pspool.tile([128, 2 * CH], f32, tag="ps", name="sB")
                    emit_s(tb[:, 0:CH], b0 + 2)
                    emit_s(tb[:, CH:], b0 + 3)
                if pending_schr is not None:
                    emit_schr_pair(pending_schr)
                    pending_schr = None
                if schr:
                    pending_schr = p

                nc.scalar.activation(e_t[:, b0 * CH:(b0 + 2) * CH], ta,
                                     AF.Exp)
                if not schr:
                    nc.scalar.activation(
                        e_t[:, (b0 + 2) * CH:(b0 + 4) * CH], tb, AF.Exp)

                for pp, hh in amult_at.get(p, ()):
                    emit_amult(e_t, ea_t, a_t, pp, hh)

                # drain previous chunk's spilled phase2 quads: 2 per slot,
                # finishing at slot 5 (12 spill quads with lag 5)
                if carry is not None and p <= 5:
                    cod, ce, cea, cch, cquads = carry
                    take = cquads[:2] if p < 5 else cquads
                    for pp, hh in take:
                        emit_ph2_quad(cod, ce, cea, pp, hh)
                    cquads = cquads[len(take):]
                    if not cquads:
                        emit_out(cod, cch)
                        carry = None
                    else:
                        carry = (cod, ce, cea, cch, cquads)

                if ch == 0:
                    if p == 1:
                        # K projection second half at slot 1 (hT half 1
                        # lands ~slot 0.5; emitting at slot 0 head-of-line
                        # blocks the PE queue on the DMA); cast on ACT so
                        # the "ps" rotation's consumers stay single-engine
                        kps1 = emit_kproj(1)
                        nc.scalar.activation(kt4[:, CH:2 * CH], kps1,
                                             AF.Copy)
                    if p < 4:
                        if p == 0:
                            vps0 = podpool.tile([128, CH], f32, tag="od",
                                                name="vps0")
                        emit_vproj_mms(vps0, 0, 2 * p, 2)
                        if p == 1:
                            emit_vproj_add(vps0, 0, 4)
                        elif p == 3:
                            emit_vproj_add(vps0, 4, 4)
                    else:
                        if p == 4:
                            vps1 = podpool.tile([128, CH], f32, tag="od",
                                                name="vps1")
                        emit_vproj_mms(vps1, 8, 2 * p, 2)
                        if p == 5:
                            emit_vproj_add(vps1, 8, 4)
                        elif p == 7:
                            emit_vproj_add(vps1, 12, 4)
                    if p == 5:
                        emit_qproj(1, on_act=True)
                elif ch in (1, 2) and p == 5:
                    emit_qproj(ch + 1, on_act=True)

                for pp, hh in ph2_at.get(p, ()):
                    if od is None:
                        od = podpool.tile([128, CH], f32, tag="od")
                    emit_ph2_quad(od, e_t, ea_t, pp, hh)

            # chunk epilogue: pair-7's deferred Schraudolph + EA
            if pending_schr is not None:
                emit_schr_pair(pending_schr)
                pending_schr = None
            for pp, hh in epilogue_amults:
                emit_amult(e_t, ea_t, a_t, pp, hh)

            carry = (od, e_t, ea_t, ch, spill)

        cod, ce, cea, cch, cquads = carry
        for p, hh in cquads:
            emit_ph2_quad(cod, ce, cea, p, hh)
        emit_out(cod, cch)

    nc.finalize()
    return nc


def kernel(h, A, Wq, bq, Wk, bk, Wv, bv):
    global LAST_RESULTS
    from concourse.bass_utils import run_bass_kernel_spmd

    h = np.asarray(h, np.float32)
    A = np.asarray(A, np.float32)
    Wq = np.asarray(Wq, np.float32)
    Wk = np.asarray(Wk, np.float32)
    Wv = np.asarray(Wv, np.float32)
    bq = np.asarray(bq, np.float32)
    bv = np.asarray(bv, np.float32)

    # hT: [b, 128(p), q(4), s(2), 512] column-blocked so each 512-col DMA
    # piece is one contiguous 2KB run per partition
    hT = (h.transpose(0, 2, 1)                     # [b, IN_DIM, N]
          .reshape(B, 2, 128, N).transpose(0, 2, 1, 3)
          .reshape(B, 128, 2, 4, 512).transpose(0, 1, 3, 2, 4)
          .reshape(B, 128, 2 * N))
    hT = np.ascontiguousarray(hT).astype(ml_dtypes.bfloat16)
    # A: [b, ch*128(p), t*CH(i)]: a_t[p, t, i] = A[b, t*128+p, ch*CH+i]
    Ab = (A.reshape(B, NJ, 128, NCH, CH).transpose(0, 3, 2, 1, 4)
          .reshape(B, NCH * 128, NJ * CH))
    Ab = np.ascontiguousarray(Ab).astype(ml_dtypes.bfloat16)
    sc = np.float32(1.0 / math.sqrt(D))

    in_maps = []
    for c in range(NCORES):
        b = c // CORES_PER_B
        h0 = HPC * (c % CORES_PER_B)
        sl = slice(h0 * D, (h0 + HPC) * D)
        wq_h = [Wq[:, (h0 + k) * D:(h0 + k + 1) * D] for k in range(HPC)]
        wq_rep = np.concatenate([wq_h[0], wq_h[0], wq_h[1], wq_h[1]], axis=1)
        bq_h = [bq[(h0 + k) * D:(h0 + k + 1) * D] for k in range(HPC)]
        bq4 = np.concatenate([bq_h[0], bq_h[0], bq_h[1], bq_h[1]]) * sc
        # one packed [128, 642] bf16 buffer: wq s0|s1, wk s0|s1, wv s0|s1
        # (each [128, m] slab), then bq4/bvb as raw f32 bytes
        slabs = []
        for w in (wq_rep, Wk[:, sl], Wv[:, sl]):
            wb = w.astype(ml_dtypes.bfloat16).view(np.uint16)
            slabs += [wb[0:128, :], wb[128:256, :]]
        slabs.append(np.ascontiguousarray(
            bq4.reshape(128, 1).astype(np.float32)).view(np.uint16))
        slabs.append(np.ascontiguousarray(
            np.tile(bv[sl][None, :], (128, 1)).astype(np.float32))
            .view(np.uint16))
        wpk = np.ascontiguousarray(
            np.concatenate(slabs, axis=1)).view(ml_dtypes.bfloat16)
        in_maps.append({
            "hT": hT[b],
            "Ab": Ab[b],
            "wpack": wpk,
        })

    nc = _build_bass()
    res = run_bass_kernel_spmd(
        nc, in_maps, core_ids=list(range(NCORES)),
        trace=os.environ.get("BASS_TRACE", "0") == "1",
    )
    LAST_RESULTS = res

    out = np.empty((B, HEADS, N, D), np.float32)
    for c in range(NCORES):
        b = c // CORES_PER_B
        h0 = HPC * (c % CORES_PER_B)
        oo = res.results[c]["o"]                  # [128, N] f32
        for hh in range(HPC):
            num = oo[hh * D:(hh + 1) * D, :]      # [32, N] unnormalized out^T
            den = oo[64 + 32 * hh, :]             # [N]
            out[b, h0 + hh] = (num / den[None, :]).T
    return out

